# revision 8
# baseline (speedup 1.0000x reference)
"""Fused multi-head self-attention (T=2048, B=2, E=1024, H=16) on 8 TRN2 cores.

Sharding: batch*heads across cores — core c handles b = c//4, heads
[(c%4)*4, (c%4)*4+4). Projections are column-split (Wq/Wk/Wv) per core's
heads; Wo is row-split with the cross-core reduction done on the host
during unshard (4 partial [T,E] sums per batch element).

Device kernel (per core, identical SPMD program):
  - q/k projections run as fp8e4 DoubleRow matmuls (K=256 per chunk,
    weights prescaled x8 on the host; the 1/8 and softmax scale fold into
    the PSUM->SBUF staging copy on ScalarE), v/out projections stay fp16
    for short-context-row accuracy
  - scores are fp8 DoubleRow too: q/k staged as fp8 [hd, m] then repacked
    by stride-2 partition DMAs into [32, 2, T] (hd = 2p+r), the two heads
    of a pair on PE row bases 0/64 so their K=64 DR streams overlap; the
    (0,0) block gets a separate fp16 corner projection + matmul so rows
    with tiny softmax support see fp16-accurate logits
  - exp on full 256-wide tiles alternates between ScalarE (native Exp ->
    fp8) and VectorE (Schraudolph: one fused mult+add fp32->int8 writes
    fp8e4m3 bits of exp(x) directly); diagonal tiles keep accurate ACT
    exp -> fp16 and fp16 AV with fp16 V strips
  - softmax reductions avoided entirely: denominators via a ones-column
    appended to each V strip (row 64 of the AV accumulation), denominator
    rows copied to SBUF p0 on ScalarE, one fast approx reciprocal,
    partition broadcast on GpSimd, one DVE multiply per head
  - causal structure via compile-time block classification: fully-masked
    blocks trimmed out of the streams, binary diagonal blocks masked
    post-exp with 0/1 multiplies on GpSimd, general additive blocks added
    pre-exp on VectorE
  - x arrives twice: fp8 (chunk-pair layout for DR projections) and fp16
    block-major (one 128-row T-block per DMA so v-projections stream
    just-in-time during the first s-loop instead of waiting for 4MB)
  - m-chunks processed [1,3,2,0]; projections/v-groups/corner emitted as
    PE filler between the exp and AV of later s-loop iterations
  - one transient-NaN retry; numpy fallback for exotic masks/key padding
"""
import os
import sys

import numpy as np

for _p in ("/opt/trn_rl_repo", "/root/.axon_site/_ro/trn_rl_repo"):
    if os.path.isdir(_p) and _p not in sys.path:
        sys.path.insert(0, _p)
        break

import ml_dtypes

import concourse.bacc as bacc
import concourse.mybir as mybir
import concourse.tile as tile
from concourse.bass_utils import run_bass_kernel_spmd

f32 = mybir.dt.float32
bf16 = mybir.dt.float16
f8 = mybir.dt.float8e4
i8 = mybir.dt.int8
AF = mybir.ActivationFunctionType
DR = mybir.MatmulPerfMode.DoubleRow
ALU = mybir.AluOpType

T, B, E, H, HD = 2048, 2, 1024, 16, 64
NCORES = 8
HL = (B * H) // NCORES          # heads per core = 4
J = HL * HD                     # per-core projection width = 256
EC = E // 128                   # fp16 e-chunks = 8
EC2 = E // 256                  # fp8 DR e-chunks = 4
NB = T // 128
SCALE = HD ** -0.5
MCH = 512                       # m-chunk width
NEG_THRESH = -1e8               # "fully masked" threshold

SKIP, ZERO, ADD, ADDBIN = 0, 1, 2, 3

# Schraudolph exp -> fp8e4m3 bits: b = 8*log2(e)*x + 8*7 (+0.5 if the
# DVE float->int8 convert truncates instead of rounding)
SCH_MUL = 11.541560327111707
SCH_BIAS = 56.5
USE_SCHRAUDOLPH = True          # fulls exp alternates DVE/ACT

_prog_cache = {}


def _classify_mask(mask):
    """Classify 128x128 blocks of mask[t_query, s_key]."""
    nb = mask.shape[0] // 128
    blocks = mask.reshape(nb, 128, nb, 128)
    all_skip = (blocks <= NEG_THRESH).all(axis=(1, 3))
    all_zero = (blocks == 0.0).all(axis=(1, 3))
    binary = ((blocks == 0.0) | (blocks <= NEG_THRESH)).all(axis=(1, 3))
    cls = np.where(all_skip, SKIP,
                   np.where(all_zero, ZERO, np.where(binary, ADDBIN, ADD)))
    return cls  # [m_block, s_block]


def _build(T_, cls_key, debug=False):
    cls = np.array(cls_key, dtype=np.int64)
    NB_ = T_ // 128
    NMC = T_ // MCH
    add_blocks = [(mb, sb) for mb in range(NB_) for sb in range(NB_)
                  if cls[mb, sb] == ADD]
    add_pos = {blk: i for i, blk in enumerate(add_blocks)}
    n_add = len(add_blocks)
    bin_blocks = [(mb, sb) for mb in range(NB_) for sb in range(NB_)
                  if cls[mb, sb] == ADDBIN]
    bin_pos = {blk: i for i, blk in enumerate(bin_blocks)}
    n_bin = len(bin_blocks)

    nc = bacc.Bacc("TRN2", target_bir_lowering=False, debug=False)
    xTb = nc.declare_dram_parameter("xTb", [128, NB_ * EC * 128], bf16,
                                    isOutput=False)
    xT8 = nc.declare_dram_parameter("xT8", [128, EC2 * 2 * T_], f8,
                                    isOutput=False)
    w8q = nc.declare_dram_parameter("w8q", [128, EC2 * 512], f8,
                                    isOutput=False)
    w8k = nc.declare_dram_parameter("w8k", [128, EC2 * 512], f8,
                                    isOutput=False)
    wqpack0 = nc.declare_dram_parameter("wqpack0", [128, EC * 128], bf16,
                                        isOutput=False)
    wqpack1 = nc.declare_dram_parameter("wqpack1", [128, EC * 128], bf16,
                                        isOutput=False)
    wkpack0 = nc.declare_dram_parameter("wkpack0", [128, EC * 128], bf16,
                                        isOutput=False)
    wkpack1 = nc.declare_dram_parameter("wkpack1", [128, EC * 128], bf16,
                                        isOutput=False)
    wvpack = nc.declare_dram_parameter("wvpack", [128, EC * J], bf16,
                                       isOutput=False)
    wopack = nc.declare_dram_parameter("wopack", [128, (J // 128) * E], bf16,
                                       isOutput=False)
    bqp = nc.declare_dram_parameter("bqp", [128, 2], f32, isOutput=False)
    msk = nc.declare_dram_parameter("msk", [128, max(n_add, 1) * 128], f32,
                                    isOutput=False)
    tri = nc.declare_dram_parameter("tri", [128, max(n_bin, 1) * 128], bf16,
                                    isOutput=False)
    out = nc.declare_dram_parameter("out", [T_, E], bf16, isOutput=True)

    with tile.TileContext(nc) as tc:
        with nc.allow_low_precision(reason="fp8/fp16 matmuls, fp32 psum"), \
             tc.tile_pool(name="sba", bufs=1) as sba, \
             tc.tile_pool(name="sbw", bufs=1) as sbw, \
             tc.tile_pool(name="ps", bufs=1, space="PSUM") as ps:
            xTb_sb = sba.tile([128, NB_ * EC * 128], bf16)
            xT8_sb = sba.tile([128, EC2 * 2 * T_], f8)
            w8q_sb = sba.tile([128, EC2 * 512], f8)
            w8k_sb = sba.tile([128, EC2 * 512], f8)
            wpack_sb = sba.tile([128, 3 * EC * J], bf16)
            wq_sb = wpack_sb[:, 0:EC * J]
            wk_sb = wpack_sb[:, EC * J:2 * EC * J]
            wv_sb = wpack_sb[:, 2 * EC * J:3 * EC * J]
            wo_sb = sba.tile([128, (J // 128) * E], bf16)
            qs8_sb = sba.tile([128, 2 * T_], f8)
            ks8_sb = sba.tile([128, 2 * T_], f8)
            q8_sb = sba.tile([128, 4 * T_], f8)
            k8_sb = sba.tile([128, 4 * T_], f8)
            qT16_sb = sba.tile([128, 256], bf16)
            kT16_sb = sba.tile([128, 256], bf16)
            v_sb = sba.tile([128, HL * NB_ * 65], bf16)
            v8_sb = sba.tile([128, (NB_ // 2) * HL * 160], f8)
            oT_sb = sba.tile([128, 2 * T_], bf16)
            bq_sb = sba.tile([128, 2], f32)
            msk_sb = sba.tile([128, max(n_add, 1) * 128], f32)
            tri_sb = sba.tile([128, max(n_bin, 1) * 128], bf16)

            # ---- input DMAs, ordered for the ramp: fp8 projection data
            # first (first matmul ~3us in), fp16 x block-major so v-groups
            # stream just-in-time during the first s-loop ----
            nc.sync.dma_start(w8k_sb[:], w8k[:, :])
            nc.sync.dma_start(w8q_sb[:], w8q[:, :])
            nc.sync.dma_start(bq_sb[:], bqp[:, :])
            for c in range(EC2):
                nc.sync.dma_start(xT8_sb[:, c * 2 * T_:(c + 1) * 2 * T_],
                                  xT8[:, c * 2 * T_:(c + 1) * 2 * T_])
            for i in range(2):
                nc.sync.dma_start(xTb_sb[:, i * 1024:(i + 1) * 1024],
                                  xTb[:, i * 1024:(i + 1) * 1024])
            nc.sync.dma_start(wv_sb[:, :], wvpack[:, :])
            for i in range(2, NB_):
                nc.sync.dma_start(xTb_sb[:, i * 1024:(i + 1) * 1024],
                                  xTb[:, i * 1024:(i + 1) * 1024])
            v_ones_view = v_sb[:].rearrange("p (x c) -> p x c", c=65)[:, :, 64:65]
            nc.vector.memset(v_ones_view, 1.0)
            v8_ones_view = v8_sb[:].rearrange("p (x c) -> p x c",
                                              c=80)[:, :, 64:65]
            nc.vector.memset(v8_ones_view, 1.0)
            nc.sync.dma_start(wo_sb[:], wopack[:, :])

            def wslc(wsb, u):
                return wsb.rearrange("p (c u x) -> p c u x", u=2,
                                     x=128)[:, :, u, :]
            nc.sync.dma_start(wslc(wq_sb, 0), wqpack0[:, :])
            nc.sync.dma_start(wslc(wq_sb, 1), wqpack1[:, :])
            nc.sync.dma_start(wslc(wk_sb, 0), wkpack0[:, :])
            nc.sync.dma_start(wslc(wk_sb, 1), wkpack1[:, :])
            if n_add:
                nc.sync.dma_start(msk_sb[:], msk[:, :])
            if n_bin:
                nc.sync.dma_start(tri_sb[:], tri[:, :])

            # ---- fp8 DR q/k projection groups + staging + repack ----
            def qk_group8(nn, u, w8sb, stag, pack, is_q):
                psq = ps.tile([128, 512], f32, tag="big", bufs=2)
                for c in range(EC2):
                    w8v = w8sb[:, c * 512 + u * 256: c * 512 + u * 256 + 256] \
                        .rearrange("p (r m) -> p r m", r=2)
                    x8v = xT8_sb[:, c * 2 * T_:(c + 1) * 2 * T_] \
                        .rearrange("p (r m) -> p r m",
                                   r=2)[:, :, nn * 512:(nn + 1) * 512]
                    nc.tensor.matmul(psq[:], w8v, x8v, start=(c == 0),
                                     stop=(c == EC2 - 1), perf_mode=DR)
                dst = stag[:, u * T_ + nn * 512: u * T_ + nn * 512 + 512]
                if is_q:
                    nc.scalar.activation(dst, psq[:], AF.Identity,
                                         bias=bq_sb[:, u:u + 1],
                                         scale=SCALE / 8.0)
                else:
                    nc.scalar.activation(dst, psq[:], AF.Copy, bias=0.0,
                                         scale=0.125)
                # repack [hd, m] -> [32, 2, m] with DR pairing hd = p+32r
                # (contiguous partition runs), heads at PE row bases 0/64,
                # pair u in its own column half
                for j in (0, 1):
                    for r in (0, 1):
                        src = stag[64 * j + 32 * r: 64 * j + 32 * r + 32,
                                   u * T_ + nn * 512: u * T_ + nn * 512 + 512]
                        dstp = pack[64 * j: 64 * j + 32,
                                    u * 2 * T_ + r * T_ + nn * 512:
                                    u * 2 * T_ + r * T_ + nn * 512 + 512]
                        nc.gpsimd.dma_start(dstp, src)

            def k_group8(nn, u):
                qk_group8(nn, u, w8k_sb, ks8_sb, k8_sb, False)

            def q_group8(nn, u):
                qk_group8(nn, u, w8q_sb, qs8_sb, q8_sb, True)

            # fp16 full-E corner projection: q[., m<128], k[., s<128] for
            # the (0,0) block's fp16 scores (short-support rows)
            def corner_group(u):
                psc = ps.tile([128, 512], f32, tag="big", bufs=2)
                for c in range(EC):
                    nc.tensor.matmul(
                        psc[:, 0:128],
                        wq_sb[:, c * J + u * 128: c * J + (u + 1) * 128],
                        xTb_sb[:, c * 128: c * 128 + 128],
                        start=(c == 0), stop=(c == EC - 1))
                for c in range(EC):
                    nc.tensor.matmul(
                        psc[:, 128:256],
                        wk_sb[:, c * J + u * 128: c * J + (u + 1) * 128],
                        xTb_sb[:, c * 128: c * 128 + 128],
                        start=(c == 0), stop=(c == EC - 1))
                nc.vector.tensor_scalar_add(qT16_sb[:, u * 128:(u + 1) * 128],
                                            psc[:, 0:128], bq_sb[:, u:u + 1])
                nc.vector.tensor_copy(kT16_sb[:, u * 128:(u + 1) * 128],
                                      psc[:, 128:256])

            def v_group(i):
                psv = ps.tile([128, 512], f32, tag="big", bufs=2)
                for c in range(EC):
                    nc.tensor.matmul(
                        psv[:, 0:J],
                        xTb_sb[:, i * 1024 + c * 128: i * 1024 + c * 128 + 128],
                        wv_sb[:, c * J:(c + 1) * J],
                        start=(c == 0), stop=(c == EC - 1))
                # ones-last strips: [v0..v63, 1] per (block, head) — the
                # denominator lands at PSUM partition 64 (legal AP base)
                dstv = v_sb[:, i * (HL * 65):(i + 1) * (HL * 65)] \
                    .rearrange("p (h c) -> p h c", c=65)[:, :, 0:64]
                srcv = psv[:, 0:J].rearrange("p (h c) -> p h c", c=64)
                nc.vector.tensor_copy(dstv, srcv)
                # fp8 strips from the fp16 strips, on Pool (SBUF-only)
                t, par = i // 2, i % 2
                dst8 = v8_sb[:, t * (HL * 160):(t + 1) * (HL * 160)] \
                    .rearrange("p (h c) -> p h c",
                               c=160)[:, :, par * 80: par * 80 + 64]
                nc.gpsimd.tensor_copy(dst8, dstv)

            from collections import deque
            order = [1, 3, 2, 0] if NMC == 4 else list(range(NMC))
            first = order[0]
            for u in range(2):
                for kk in range(first + 1):
                    k_group8(kk, u)
                q_group8(first, u)
            for i in range(2):
                v_group(i)
            vdone = 2

            def _k_thunk(nn, u):
                return lambda: k_group8(nn, u)

            def _q_thunk(nn, u):
                return lambda: q_group8(nn, u)

            def _c_thunk(u):
                return lambda: corner_group(u)

            def _v_thunk(i):
                return lambda: v_group(i)

            fill = deque()
            need0 = min(4 * first + 4, NB_)
            for i in range(vdone, need0):
                fill.append((1, _v_thunk(i)))
            vdone = need0
            maxk = first
            for pos in range(1, NMC):
                nn = order[pos]
                for u in range(2):
                    for kk in range(maxk + 1, nn + 1):
                        fill.append((pos, _k_thunk(kk, u)))
                    fill.append((pos, _q_thunk(nn, u)))
                    if nn == 0:
                        fill.append((pos, _c_thunk(u)))
                maxk = max(maxk, nn)
                need = min(4 * nn + 4, NB_)
                for i in range(vdone, need):
                    fill.append((pos, _v_thunk(i)))
                vdone = max(vdone, need)
            for i in range(vdone, NB_):
                fill.append((NMC - 1, _v_thunk(i)))

            exp_cnt = [0]

            # ---- attention: DR scores (heads on PE halves 0/64), exp
            # split DVE-Schraudolph/ACT, split-K AV, pipelined normalize,
            # deferred out-proj ----
            def s_loop_pair(n, u, side_work=(), fill_q=None):
                side_work = list(side_work)
                hA, hB = 2 * u, 2 * u + 1
                ub = u * 2 * T_

                def slot(pss_cur=None):
                    if side_work:
                        side_work.pop(0)()
                        if fill_q:
                            fill_q.popleft()[1]()
                            if fill_q:
                                fill_q.popleft()[1]()
                    elif fill_q:
                        fill_q.popleft()[1]()
                        if fill_q:
                            fill_q.popleft()[1]()

                k8A = k8_sb[0:32, ub:ub + 2 * T_] \
                    .rearrange("p (r s) -> p r s", r=2)
                k8B = k8_sb[64:96, ub:ub + 2 * T_] \
                    .rearrange("p (r s) -> p r s", r=2)
                q8A = q8_sb[0:32, ub:ub + 2 * T_] \
                    .rearrange("p (r m) -> p r m", r=2)
                q8B = q8_sb[64:96, ub:ub + 2 * T_] \
                    .rearrange("p (r m) -> p r m", r=2)

                full_t = [t for t in range(NB_ // 2)
                          if all(cls[n * 4 + k, 2 * t + s] == ZERO
                                 for k in range(4) for s in (0, 1))]
                dr_cov = {i for t in full_t for i in (2 * t, 2 * t + 1)}
                stiles = [i for i in range(NB_) if i not in dr_cov
                          and any(cls[n * 4 + k, i] != SKIP for k in range(4))]
                psoA = ps.tile([128, 512], f32, tag="attno", bufs=4)
                psoB = ps.tile([128, 512], f32, tag="attno", bufs=4)
                for ti, t in enumerate(full_t):
                    pt8 = sbw.tile([128, 2048], f8, tag="pt8", bufs=4)
                    pt8i = pt8.bitcast(i8)
                    for sub in (0, 1):
                        i = 2 * t + sub
                        pss = ps.tile([128, 1024], f32, tag="big", bufs=2)
                        nc.tensor.matmul(
                            pss[:, 0:512], k8A[:, :, i * 128:(i + 1) * 128],
                            q8A[:, :, n * 512:(n + 1) * 512], start=True,
                            stop=True, perf_mode=DR, skip_group_check=True)
                        nc.tensor.matmul(
                            pss[:, 512:1024], k8B[:, :, i * 128:(i + 1) * 128],
                            q8B[:, :, n * 512:(n + 1) * 512], start=True,
                            stop=True, perf_mode=DR, skip_group_check=True)
                        if USE_SCHRAUDOLPH and exp_cnt[0] % 2 == 0:
                            nc.vector.tensor_scalar(
                                pt8i[:, sub * 1024:(sub + 1) * 1024], pss[:],
                                SCH_MUL, SCH_BIAS, ALU.mult, ALU.add)
                        else:
                            nc.scalar.activation(
                                pt8[:, sub * 1024:(sub + 1) * 1024], pss[:],
                                AF.Exp)
                        exp_cnt[0] += 1
                        slot(pss)
                    last_here = (ti == len(full_t) - 1) and not stiles
                    pt83 = pt8[:].rearrange("p (s x) -> p s x", x=1024)
                    for pso_, h, off in ((psoA, hA, 0), (psoB, hB, 512)):
                        v8v = v8_sb[:, t * (HL * 160) + h * 160:
                                    t * (HL * 160) + (h + 1) * 160] \
                            .rearrange("p (o c) -> p o c", c=80)[:, :, 0:65]
                        nc.tensor.matmul(
                            pso_[0:65, :], v8v, pt83[:, :, off:off + 512],
                            start=(ti == 0), stop=last_here,
                            perf_mode=DR, skip_group_check=True)
                dr_started = bool(full_t)
                last = len(stiles) - 1
                for idx, i in enumerate(stiles):
                    runs = []
                    k = 0
                    while k < 4:
                        k1 = k
                        skipk = cls[n * 4 + k, i] == SKIP
                        while k1 < 4 and (cls[n * 4 + k1, i] == SKIP) == skipk:
                            k1 += 1
                        runs.append((k, k1, skipk))
                        k = k1
                    if runs[0][2]:
                        w0 = runs[0][1] * 128
                        runs = runs[1:]
                    else:
                        w0 = 0
                    pss = ps.tile([128, 1024], f32, tag="big", bufs=2)
                    corner = (n == 0 and i == 0)
                    a0 = 128 if corner else w0
                    if corner:
                        nc.tensor.matmul(
                            pss[:, 0:128], kT16_sb[0:64, u * 128:(u + 1) * 128],
                            qT16_sb[0:64, u * 128:(u + 1) * 128],
                            start=True, stop=True, skip_group_check=True)
                        nc.tensor.matmul(
                            pss[:, 512:640],
                            kT16_sb[64:128, u * 128:(u + 1) * 128],
                            qT16_sb[64:128, u * 128:(u + 1) * 128],
                            start=True, stop=True, skip_group_check=True)
                    nc.tensor.matmul(
                        pss[:, a0:512], k8A[:, :, i * 128:(i + 1) * 128],
                        q8A[:, :, n * 512 + a0:(n + 1) * 512], start=True,
                        stop=True, perf_mode=DR, skip_group_check=True)
                    nc.tensor.matmul(
                        pss[:, 512 + a0:1024], k8B[:, :, i * 128:(i + 1) * 128],
                        q8B[:, :, n * 512 + a0:(n + 1) * 512], start=True,
                        stop=True, perf_mode=DR, skip_group_check=True)
                    pss3 = pss[:].rearrange("p (o w) -> p o w", w=512)
                    for k in range(4):
                        if cls[n * 4 + k, i] == ADD:
                            mpos = add_pos[(n * 4 + k, i)]
                            mblk = msk_sb[:, mpos * 128:(mpos + 1) * 128]
                            mblk3 = mblk.unsqueeze(1).broadcast_to([128, 2, 128])
                            nc.vector.tensor_add(
                                pss3[:, :, k * 128:(k + 1) * 128],
                                pss3[:, :, k * 128:(k + 1) * 128],
                                mblk3)
                    pt = sbw.tile([128, 1024], bf16, tag="pt", bufs=6)
                    pt3 = pt[:].rearrange("p (o w) -> p o w", w=512)
                    if runs == [(0, 4, False)]:
                        nc.scalar.activation(pt[:], pss[:], AF.Exp)
                    else:
                        for k, k1, skipk in runs:
                            a = max(k * 128, w0)
                            src = pss3[:, :, a: k1 * 128]
                            dst = pt3[:, :, a: k1 * 128]
                            if skipk:
                                nc.gpsimd.memset(dst, 0.0)
                            else:
                                nc.scalar.activation(dst, src, AF.Exp)
                    for k in range(4):
                        if cls[n * 4 + k, i] == ADDBIN:
                            tpos = bin_pos[(n * 4 + k, i)]
                            tblk = tri_sb[:, tpos * 128:(tpos + 1) * 128]
                            tblk3 = tblk.unsqueeze(1).broadcast_to([128, 2, 128])
                            nc.gpsimd.tensor_mul(
                                pt3[:, :, k * 128:(k + 1) * 128],
                                pt3[:, :, k * 128:(k + 1) * 128],
                                tblk3)
                    slot(pss)
                    for pso_, h, off in ((psoA, hA, 0), (psoB, hB, 512)):
                        strip = v_sb[:, i * (HL * 65) + h * 65:
                                     i * (HL * 65) + h * 65 + 65]
                        nc.tensor.matmul(
                            pso_[0:65, w0:512], strip[:, :],
                            pt[:, off + w0:off + 512],
                            start=(idx == 0 and not dr_started),
                            stop=(idx == last),
                            skip_group_check=True)
                while side_work:
                    side_work.pop(0)()
                return psoA, psoB

            def normalize_pair(n, u, psoA, psoB):
                """Denominator rows (PSUM partition 64) copied to SBUF p0
                on ScalarE (reciprocal_approx_fast misreads PSUM base 64),
                one fast approx reciprocal over both heads, partition
                broadcast on Pool, one DVE multiply per head."""
                recd = sbw.tile([1, 1024], f32, tag="recd", bufs=3)
                recf = sbw.tile([1, 1024], f32, tag="recf", bufs=3)
                rb = sbw.tile([64, 1024], f32, tag="rb", bufs=2)
                col = u * T_ + n * 512

                def cA():
                    nc.scalar.copy(recd[0:1, 0:512], psoA[64:65, :])

                def cB():
                    nc.scalar.copy(recd[0:1, 512:1024], psoB[64:65, :])

                def rr():
                    nc.vector.reciprocal_approx_fast(recf[:], recd[:])

                def pb():
                    nc.gpsimd.partition_broadcast(rb[:, :], recf[:, :])

                def mA():
                    nc.vector.tensor_mul(oT_sb[0:64, col:col + 512],
                                         psoA[0:64, :], rb[0:64, 0:512])

                def mB():
                    nc.vector.tensor_mul(oT_sb[64:128, col:col + 512],
                                         psoB[0:64, :], rb[0:64, 512:1024])
                return [cA, cB, rr, pb, mA, mB]

            def out_proj_group(m16, eh):
                pso = ps.tile([128, 512], f32, tag="big", bufs=2)
                for jc in range(J // 128):
                    nc.tensor.matmul(
                        pso[:],
                        oT_sb[:, jc * T_ + m16 * 128: jc * T_ + m16 * 128 + 128],
                        wo_sb[:, jc * E + eh * 512: jc * E + eh * 512 + 512],
                        start=(jc == 0), stop=(jc == J // 128 - 1),
                        skip_group_check=True)
                ob = sbw.tile([128, 512], bf16, tag="ob", bufs=6)
                if (m16 + eh) % 2 == 0:
                    nc.scalar.copy(ob[:], pso[:])
                else:
                    nc.vector.tensor_copy(ob[:], pso[:])
                nc.gpsimd.dma_start(
                    out[m16 * 128:(m16 + 1) * 128,
                        eh * 512:(eh + 1) * 512], ob[:])

            def out_proj_thunks(nn):
                gs = [(m16, eh) for m16 in range(nn * 4, nn * 4 + 4)
                      for eh in range(E // 512)]

                def duo(a, b):
                    def go():
                        out_proj_group(a[0], a[1])
                        out_proj_group(b[0], b[1])
                    return go
                return [duo(gs[i], gs[i + 1]) for i in range(0, len(gs), 2)]

            def out_proj(nn):
                for w in out_proj_thunks(nn):
                    w()

            prevpair = None
            carry = []
            last_op = []
            for pos in range(NMC):
                n = order[pos]
                for u in range(2):
                    work = []
                    if prevpair is not None:
                        ppos, pn, pu, pA, pB = prevpair
                        work = normalize_pair(pn, pu, pA, pB)
                    work += carry
                    carry = []
                    while fill and fill[0][0] <= pos:
                        fill.popleft()[1]()
                    psoA, psoB = s_loop_pair(n, u, work, fill)
                    if prevpair is not None and pu == 1:
                        if ppos == NMC - 2:
                            last_op = out_proj_thunks(pn)
                        else:
                            carry = out_proj_thunks(pn)
                    prevpair = (pos, n, u, psoA, psoB)
            for w in carry:
                w()
            ppos, pn, pu, pA, pB = prevpair
            wAB = normalize_pair(pn, pu, pA, pB)
            lo = last_op if NMC >= 2 else []
            for idx in range(max(len(wAB), len(lo))):
                if idx < len(wAB):
                    wAB[idx]()
                if idx < len(lo):
                    lo[idx]()
            out_proj(pn)

    nc.compile()
    return nc


def _get_program(T_, cls):
    key = (T_, tuple(map(tuple, cls.tolist())))
    if key not in _prog_cache:
        _prog_cache[key] = _build(T_, key[1])
    return _prog_cache[key]


def _numpy_ref(query, attn_mask, key_padding_mask, Wq, bq, Wk, bk, Wv, bv,
               Wo, bo):
    """Exact-semantics fallback (mirrors reference.py in numpy)."""
    q = (query @ Wq.T + bq) * SCALE
    k = query @ Wk.T + bk
    v = query @ Wv.T + bv

    def shp(x):
        return x.reshape(T, B * H, HD).transpose(1, 0, 2)

    q, k, v = shp(q), shp(k), shp(v)
    w = np.einsum('bth,bsh->bts', q, k).reshape(B, H, T, T) + attn_mask
    w = np.where(key_padding_mask[:, None, None, :], -np.inf, w)
    w = w - w.max(axis=-1, keepdims=True)
    ew = np.exp(w)
    p = (ew / ew.sum(axis=-1, keepdims=True)).reshape(B * H, T, T)
    o = np.einsum('bts,bsh->bth', p, v.reshape(B * H, T, HD))
    o = o.transpose(1, 0, 2).reshape(T, B, E)
    return (o @ Wo.T + bo).astype(np.float32)


def _prep_inputs(query, attn_mask, Wq, bq, Wk, Wv, Wo, cls):
    """Build the 8 per-core input maps."""
    bf = np.float16
    f8np = ml_dtypes.float8_e4m3
    add_blocks = [(mb, sb) for mb in range(T // 128) for sb in range(T // 128)
                  if cls[mb, sb] == ADD]
    n_add = len(add_blocks)
    if n_add:
        mskp = np.empty((128, n_add * 128), np.float32)
        for i, (mb, sb) in enumerate(add_blocks):
            blk = attn_mask[mb * 128:(mb + 1) * 128, sb * 128:(sb + 1) * 128]
            mskp[:, i * 128:(i + 1) * 128] = np.ascontiguousarray(blk.T)
    else:
        mskp = np.zeros((128, 128), np.float32)
    bin_blocks = [(mb, sb) for mb in range(T // 128) for sb in range(T // 128)
                  if cls[mb, sb] == ADDBIN]
    if bin_blocks:
        trip = np.empty((128, len(bin_blocks) * 128), bf)
        for i, (mb, sb) in enumerate(bin_blocks):
            blk = attn_mask[mb * 128:(mb + 1) * 128, sb * 128:(sb + 1) * 128]
            trip[:, i * 128:(i + 1) * 128] = (blk.T == 0.0).astype(bf)
    else:
        trip = np.zeros((128, 128), bf)

    in_maps = []
    for core in range(NCORES):
        b = core // (NCORES // B)
        jsl = slice((core % (NCORES // B)) * J, (core % (NCORES // B)) * J + J)
        EC_, J_ = E // 128, J

        x = np.ascontiguousarray(query[:, b, :].T)  # [E, T] f32
        xTb_c = np.ascontiguousarray(
            x.reshape(EC_, 128, NB, 128).transpose(1, 2, 0, 3)
            .reshape(128, NB * EC_ * 128)).astype(bf)
        xT8_c = np.ascontiguousarray(
            x.reshape(EC2, 2, 128, T).transpose(2, 0, 1, 3)
            .reshape(128, EC2 * 2 * T)).astype(f8np)

        def w8pack(W):
            wt = W[jsl, :].T * np.float32(8.0)  # [E, J], x8 prescale
            return np.ascontiguousarray(
                wt.reshape(EC2, 2, 128, 2, 128).transpose(2, 0, 3, 1, 4)
                .reshape(128, EC2 * 2 * 2 * 128)).astype(f8np)

        def sb_layout(wT):  # [E, J] -> SBUF [128, EC*J]
            return np.ascontiguousarray(
                wT.reshape(EC_, 128, J_).transpose(1, 0, 2)
                .reshape(128, EC_ * J_))

        wq_l = sb_layout((Wq[jsl, :] * np.float32(SCALE)).T)
        wk_l = sb_layout(Wk[jsl, :].T)
        wv_l = sb_layout(Wv[jsl, :].T)

        def usplit(wl, u):  # [128, EC*J] -> u-half [128, EC*128]
            return np.ascontiguousarray(
                wl.reshape(128, EC_, 2, 128)[:, :, u, :]
                .reshape(128, EC_ * 128)).astype(bf)

        woT = Wo[:, jsl].T  # [J, E]
        wopack = np.ascontiguousarray(
            woT.reshape(J_ // 128, 128, E).transpose(1, 0, 2)
            .reshape(128, (J_ // 128) * E)).astype(bf)
        bq_c = np.ascontiguousarray(
            (bq[jsl] * np.float32(SCALE)).reshape(2, 128).T)
        in_maps.append({
            "xTb": xTb_c, "xT8": xT8_c,
            "w8q": w8pack(Wq), "w8k": w8pack(Wk),
            "wqpack0": usplit(wq_l, 0), "wqpack1": usplit(wq_l, 1),
            "wkpack0": usplit(wk_l, 0), "wkpack1": usplit(wk_l, 1),
            "wvpack": np.ascontiguousarray(wv_l).astype(bf),
            "wopack": wopack, "bqp": bq_c, "msk": mskp, "tri": trip,
        })
    return in_maps


def _kernel_impl(inputs, trace=False, **run_kwargs):
    query = np.asarray(inputs["query"], np.float32)
    attn_mask = np.asarray(inputs["attn_mask"], np.float32)
    kpm = np.asarray(inputs["key_padding_mask"])
    Wq = np.asarray(inputs["Wq"], np.float32)
    bq = np.asarray(inputs["bq"], np.float32)
    Wk = np.asarray(inputs["Wk"], np.float32)
    bk = np.asarray(inputs["bk"], np.float32)
    Wv = np.asarray(inputs["Wv"], np.float32)
    bv = np.asarray(inputs["bv"], np.float32)
    Wo = np.asarray(inputs["Wo"], np.float32)
    bo = np.asarray(inputs["bo"], np.float32)

    cls = _classify_mask(attn_mask)
    fallback = (
        kpm.any()
        or (attn_mask.max(axis=1) <= NEG_THRESH).any()
        or (cls == ADD).sum() > 24 or (cls == ADDBIN).sum() > 24
        or np.isnan(attn_mask).any()
    )
    if fallback:
        return _numpy_ref(query, attn_mask, kpm, Wq, bq, Wk, bk, Wv, bv,
                          Wo, bo), None

    nc = _get_program(T, cls)
    in_maps = _prep_inputs(query, attn_mask, Wq, bq, Wk, Wv, Wo, cls)
    for attempt in range(3):
        res = run_bass_kernel_spmd(nc, in_maps, core_ids=list(range(NCORES)),
                                   trace=trace, **run_kwargs)
        if all(np.isfinite(r["out"]).all() for r in res.results):
            break
    else:
        return _numpy_ref(query, attn_mask, kpm, Wq, bq, Wk, bk, Wv, bv,
                          Wo, bo), None

    # unshard: sum the 4 row-split partials per batch element (the Wo
    # all-reduce), then add bo and the bv contribution (sum_s p = 1).
    bo_total = bo + Wo @ bv
    out = np.empty((T, B, E), np.float32)
    gsz = NCORES // B
    for b in range(B):
        acc = res.results[b * gsz]["out"].astype(np.float32)
        for c in range(b * gsz + 1, (b + 1) * gsz):
            acc = acc + res.results[c]["out"].astype(np.float32)
        out[:, b, :] = acc + bo_total[None, :]
    return out, res


def kernel(**inputs):
    out, _ = _kernel_impl(inputs, trace=False)
    return out


# revision 17
# speedup vs baseline: 1.3141x; 1.3141x over previous
"""Fused multi-head self-attention (T=2048, B=2, E=1024, H=16) on 8 TRN2 cores.

Sharding: batch*heads across cores — core c handles b = c//4, heads
[(c%4)*4, (c%4)*4+4). Projections are column-split (Wq/Wk/Wv) per core's
heads; Wo is row-split with the cross-core reduction done on the host
during unshard (4 partial [T,E] sums per batch element).

Device kernel (per core, identical SPMD program):
  - q/k projections run as fp8e4 DoubleRow matmuls (K=256 per chunk, 2x
    fewer streams than fp16; weights prescaled x8 on the host; the 1/8
    and softmax scale fold into the PSUM->SBUF copy, split ACT/DVE) into
    fp16 qT/kT; v/out projections stay fp16 for accuracy
  - scores stay fp16 (DoubleRow streams at 1 cycle/output column, so the
    2-head-concurrent fp16 path is already at the PE scores floor); a
    full-E fp16 corner projection overwrites qT/kT cols 0:128 so the
    (0,0) block's tiny-softmax-support rows see fp16-accurate logits
  - exp on full 256-wide tiles alternates between ScalarE (native Exp ->
    fp8) and VectorE (Schraudolph: one fused mult+add fp32->int8 writes
    fp8e4m3 bits of exp(x) directly); diagonal tiles keep accurate ACT
    exp -> fp16 and fp16 AV with fp16 V strips
  - softmax reductions avoided entirely: denominators via a ones-column
    appended to each V strip (row 64 of the AV accumulation), denominator
    rows copied to SBUF p0 on ScalarE, one fast approx reciprocal,
    partition broadcast on GpSimd, one DVE multiply per head
  - causal structure via compile-time block classification: fully-masked
    blocks trimmed out of the streams, binary diagonal blocks masked
    post-exp with 0/1 multiplies on GpSimd, general additive blocks added
    pre-exp on VectorE
  - x arrives twice: fp8 (chunk-pair layout for DR projections) and fp16
    block-major (one 128-row T-block per DMA so v-projections stream
    just-in-time during the first s-loop instead of waiting for 4MB)
  - m-chunks processed [1,3,2,0]; projections/v-groups/corner emitted as
    PE filler between the exp and AV of later s-loop iterations
  - one transient-NaN retry; numpy fallback for exotic masks/key padding
"""
import os
import sys

import numpy as np

for _p in ("/opt/trn_rl_repo", "/root/.axon_site/_ro/trn_rl_repo"):
    if os.path.isdir(_p) and _p not in sys.path:
        sys.path.insert(0, _p)
        break

import ml_dtypes

import concourse.bacc as bacc
import concourse.mybir as mybir
import concourse.tile as tile
from concourse.bass_utils import run_bass_kernel_spmd

f32 = mybir.dt.float32
bf16 = mybir.dt.float16
f8 = mybir.dt.float8e4
i8 = mybir.dt.int8
AF = mybir.ActivationFunctionType
DR = mybir.MatmulPerfMode.DoubleRow
ALU = mybir.AluOpType

T, B, E, H, HD = 2048, 2, 1024, 16, 64
NCORES = 8
HL = (B * H) // NCORES          # heads per core = 4
J = HL * HD                     # per-core projection width = 256
EC = E // 128                   # fp16 e-chunks = 8
EC2 = E // 256                  # fp8 DR e-chunks = 4
NB = T // 128
SCALE = HD ** -0.5
MCH = 512                       # m-chunk width
NEG_THRESH = -1e8               # "fully masked" threshold

SKIP, ZERO, ADD, ADDBIN = 0, 1, 2, 3

# Schraudolph exp -> fp8e4m3 bits: b = 8*log2(e)*x + 8*7 (+0.5 if the
# DVE float->int8 convert truncates instead of rounding)
SCH_MUL = 11.541560327111707
SCH_BIAS = 56.5
USE_SCHRAUDOLPH = True          # fulls exp alternates DVE/ACT

_prog_cache = {}


def _classify_mask(mask):
    """Classify 128x128 blocks of mask[t_query, s_key]."""
    nb = mask.shape[0] // 128
    blocks = mask.reshape(nb, 128, nb, 128)
    all_skip = (blocks <= NEG_THRESH).all(axis=(1, 3))
    all_zero = (blocks == 0.0).all(axis=(1, 3))
    binary = ((blocks == 0.0) | (blocks <= NEG_THRESH)).all(axis=(1, 3))
    cls = np.where(all_skip, SKIP,
                   np.where(all_zero, ZERO, np.where(binary, ADDBIN, ADD)))
    return cls  # [m_block, s_block]


def _build(T_, cls_key, debug=False):
    cls = np.array(cls_key, dtype=np.int64)
    NB_ = T_ // 128
    NMC = T_ // MCH
    add_blocks = [(mb, sb) for mb in range(NB_) for sb in range(NB_)
                  if cls[mb, sb] == ADD]
    add_pos = {blk: i for i, blk in enumerate(add_blocks)}
    n_add = len(add_blocks)
    bin_blocks = [(mb, sb) for mb in range(NB_) for sb in range(NB_)
                  if cls[mb, sb] == ADDBIN]
    bin_pos = {blk: i for i, blk in enumerate(bin_blocks)}
    n_bin = len(bin_blocks)

    nc = bacc.Bacc("TRN2", target_bir_lowering=False, debug=False)
    xTb = nc.declare_dram_parameter("xTb", [128, NB_ * EC * 128], bf16,
                                    isOutput=False)
    xT8 = nc.declare_dram_parameter("xT8", [128, EC2 * 2 * T_], f8,
                                    isOutput=False)
    w8q = nc.declare_dram_parameter("w8q", [128, EC2 * 512], f8,
                                    isOutput=False)
    w8k = nc.declare_dram_parameter("w8k", [128, EC2 * 512], f8,
                                    isOutput=False)
    wqpack0 = nc.declare_dram_parameter("wqpack0", [128, EC * 128], bf16,
                                        isOutput=False)
    wqpack1 = nc.declare_dram_parameter("wqpack1", [128, EC * 128], bf16,
                                        isOutput=False)
    wkpack0 = nc.declare_dram_parameter("wkpack0", [128, EC * 128], bf16,
                                        isOutput=False)
    wkpack1 = nc.declare_dram_parameter("wkpack1", [128, EC * 128], bf16,
                                        isOutput=False)
    wvpack = nc.declare_dram_parameter("wvpack", [128, EC * J], bf16,
                                       isOutput=False)
    wopack = nc.declare_dram_parameter("wopack", [128, (J // 128) * E], bf16,
                                       isOutput=False)
    bqp = nc.declare_dram_parameter("bqp", [128, 2], f32, isOutput=False)
    msk = nc.declare_dram_parameter("msk", [128, max(n_add, 1) * 128], f32,
                                    isOutput=False)
    tri = nc.declare_dram_parameter("tri", [128, max(n_bin, 1) * 128], bf16,
                                    isOutput=False)
    out = nc.declare_dram_parameter("out", [T_, E], bf16, isOutput=True)

    with tile.TileContext(nc) as tc:
        with nc.allow_low_precision(reason="fp8/fp16 matmuls, fp32 psum"), \
             tc.tile_pool(name="sba", bufs=1) as sba, \
             tc.tile_pool(name="sbw", bufs=1) as sbw, \
             tc.tile_pool(name="ps", bufs=1, space="PSUM") as ps:
            xTb_sb = sba.tile([128, NB_ * EC * 128], bf16)
            xT8_sb = sba.tile([128, EC2 * 2 * T_], f8)
            w8q_sb = sba.tile([128, EC2 * 512], f8)
            w8k_sb = sba.tile([128, EC2 * 512], f8)
            wpack_sb = sba.tile([128, 3 * EC * J], bf16)
            wq_sb = wpack_sb[:, 0:EC * J]
            wk_sb = wpack_sb[:, EC * J:2 * EC * J]
            wv_sb = wpack_sb[:, 2 * EC * J:3 * EC * J]
            wo_sb = sba.tile([128, (J // 128) * E], bf16)
            qT_sb = sba.tile([128, 2 * T_], bf16)
            kT_sb = sba.tile([128, 2 * T_], bf16)
            v_sb = sba.tile([128, HL * NB_ * 65], bf16)
            v8_sb = sba.tile([128, (NB_ // 2) * HL * 160], f8)
            oT_sb = sba.tile([128, 2 * T_], bf16)
            bq_sb = sba.tile([128, 2], f32)
            msk_sb = sba.tile([128, max(n_add, 1) * 128], f32)
            tri_sb = sba.tile([128, max(n_bin, 1) * 128], bf16)

            # ---- input DMAs, ordered for the ramp: fp8 projection data
            # first (first matmul ~3us in), fp16 x block-major so v-groups
            # stream just-in-time during the first s-loop ----
            nc.sync.dma_start(w8k_sb[:], w8k[:, :])
            nc.sync.dma_start(w8q_sb[:], w8q[:, :])
            nc.sync.dma_start(bq_sb[:], bqp[:, :])
            for c in range(EC2):
                nc.sync.dma_start(xT8_sb[:, c * 2 * T_:(c + 1) * 2 * T_],
                                  xT8[:, c * 2 * T_:(c + 1) * 2 * T_])
            for i in range(2):
                nc.sync.dma_start(xTb_sb[:, i * 1024:(i + 1) * 1024],
                                  xTb[:, i * 1024:(i + 1) * 1024])
            nc.sync.dma_start(wv_sb[:, :], wvpack[:, :])
            for i in range(2, NB_):
                nc.sync.dma_start(xTb_sb[:, i * 1024:(i + 1) * 1024],
                                  xTb[:, i * 1024:(i + 1) * 1024])
            v_ones_view = v_sb[:].rearrange("p (x c) -> p x c", c=65)[:, :, 64:65]
            nc.vector.memset(v_ones_view, 1.0)
            v8_ones_view = v8_sb[:].rearrange("p (x c) -> p x c",
                                              c=80)[:, :, 64:65]
            nc.vector.memset(v8_ones_view, 1.0)
            nc.sync.dma_start(wo_sb[:], wopack[:, :])

            def wslc(wsb, u):
                return wsb.rearrange("p (c u x) -> p c u x", u=2,
                                     x=128)[:, :, u, :]
            nc.sync.dma_start(wslc(wq_sb, 0), wqpack0[:, :])
            nc.sync.dma_start(wslc(wq_sb, 1), wqpack1[:, :])
            nc.sync.dma_start(wslc(wk_sb, 0), wkpack0[:, :])
            nc.sync.dma_start(wslc(wk_sb, 1), wkpack1[:, :])
            if n_add:
                nc.sync.dma_start(msk_sb[:], msk[:, :])
            if n_bin:
                nc.sync.dma_start(tri_sb[:], tri[:, :])

            # ---- fp8 DR q/k projection groups -> fp16 qT/kT staging ----
            qk_cnt = [0]

            def qk_group8(nn, u, w8sb, stag, is_q):
                psq = ps.tile([128, 512], f32, tag="big", bufs=2)
                for c in range(EC2):
                    w8v = w8sb[:, c * 512 + u * 256: c * 512 + u * 256 + 256] \
                        .rearrange("p (r m) -> p r m", r=2)
                    x8v = xT8_sb[:, c * 2 * T_:(c + 1) * 2 * T_] \
                        .rearrange("p (r m) -> p r m",
                                   r=2)[:, :, nn * 512:(nn + 1) * 512]
                    nc.tensor.matmul(psq[:], w8v, x8v, start=(c == 0),
                                     stop=(c == EC2 - 1), perf_mode=DR)
                dst = stag[:, u * T_ + nn * 512: u * T_ + nn * 512 + 512]
                on_act = qk_cnt[0] % 2 == 0
                qk_cnt[0] += 1
                if is_q:
                    if on_act:
                        nc.scalar.activation(dst, psq[:], AF.Identity,
                                             bias=bq_sb[:, u:u + 1],
                                             scale=SCALE / 8.0)
                    else:
                        nc.vector.tensor_scalar(dst, psq[:], SCALE / 8.0,
                                                bq_sb[:, u:u + 1], ALU.mult,
                                                ALU.add)
                else:
                    if on_act:
                        nc.scalar.activation(dst, psq[:], AF.Copy, bias=0.0,
                                             scale=0.125)
                    else:
                        nc.vector.tensor_scalar_mul(dst, psq[:], 0.125)

            def k_group8(nn, u):
                qk_group8(nn, u, w8k_sb, kT_sb, False)

            def q_group8(nn, u):
                qk_group8(nn, u, w8q_sb, qT_sb, True)

            # fp16 full-E corner projection overwrites qT/kT cols 0:128 so
            # the (0,0) block's short-support rows see fp16 logits
            def corner_group(u):
                psc = ps.tile([128, 512], f32, tag="big", bufs=2)
                for c in range(EC):
                    nc.tensor.matmul(
                        psc[:, 0:128],
                        wq_sb[:, c * J + u * 128: c * J + (u + 1) * 128],
                        xTb_sb[:, c * 128: c * 128 + 128],
                        start=(c == 0), stop=(c == EC - 1))
                for c in range(EC):
                    nc.tensor.matmul(
                        psc[:, 128:256],
                        wk_sb[:, c * J + u * 128: c * J + (u + 1) * 128],
                        xTb_sb[:, c * 128: c * 128 + 128],
                        start=(c == 0), stop=(c == EC - 1))
                nc.vector.tensor_scalar_add(qT_sb[:, u * T_: u * T_ + 128],
                                            psc[:, 0:128], bq_sb[:, u:u + 1])
                nc.vector.tensor_copy(kT_sb[:, u * T_: u * T_ + 128],
                                      psc[:, 128:256])

            def v_group(i):
                psv = ps.tile([128, 512], f32, tag="big", bufs=2)
                for c in range(EC):
                    nc.tensor.matmul(
                        psv[:, 0:J],
                        xTb_sb[:, i * 1024 + c * 128: i * 1024 + c * 128 + 128],
                        wv_sb[:, c * J:(c + 1) * J],
                        start=(c == 0), stop=(c == EC - 1))
                # ones-last strips: [v0..v63, 1] per (block, head) — the
                # denominator lands at PSUM partition 64 (legal AP base)
                dstv = v_sb[:, i * (HL * 65):(i + 1) * (HL * 65)] \
                    .rearrange("p (h c) -> p h c", c=65)[:, :, 0:64]
                srcv = psv[:, 0:J].rearrange("p (h c) -> p h c", c=64)
                nc.vector.tensor_copy(dstv, srcv)
                # fp8 strips straight from PSUM on DVE
                t, par = i // 2, i % 2
                dst8 = v8_sb[:, t * (HL * 160):(t + 1) * (HL * 160)] \
                    .rearrange("p (h c) -> p h c",
                               c=160)[:, :, par * 80: par * 80 + 64]
                nc.vector.tensor_copy(dst8, srcv)

            from collections import deque
            order = [1, 3, 2, 0] if NMC == 4 else list(range(NMC))
            first = order[0]
            for u in range(2):
                for kk in range(first + 1):
                    k_group8(kk, u)
                q_group8(first, u)
            for i in range(2):
                v_group(i)
            vdone = 2

            def _k_thunk(nn, u):
                return lambda: k_group8(nn, u)

            def _q_thunk(nn, u):
                return lambda: q_group8(nn, u)

            def _c_thunk(u):
                return lambda: corner_group(u)

            def _v_thunk(i):
                return lambda: v_group(i)

            fill = deque()
            need0 = min(4 * first + 4, NB_)
            for i in range(vdone, need0):
                fill.append((1, _v_thunk(i)))
            vdone = need0
            maxk = first
            for pos in range(1, NMC):
                nn = order[pos]
                for u in range(2):
                    for kk in range(maxk + 1, nn + 1):
                        fill.append((pos, _k_thunk(kk, u)))
                    fill.append((pos, _q_thunk(nn, u)))
                    if nn == 0:
                        fill.append((pos, _c_thunk(u)))
                maxk = max(maxk, nn)
                need = min(4 * nn + 4, NB_)
                for i in range(vdone, need):
                    fill.append((pos, _v_thunk(i)))
                vdone = max(vdone, need)
            for i in range(vdone, NB_):
                fill.append((NMC - 1, _v_thunk(i)))

            exp_cnt = [0]

            # ---- attention: DR scores (heads on PE halves 0/64), exp
            # split DVE-Schraudolph/ACT, split-K AV, pipelined normalize,
            # deferred out-proj ----
            def s_loop_pair(n, u, side_work=(), fill_q=None):
                side_work = list(side_work)
                hA, hB = 2 * u, 2 * u + 1

                def slot(pss_cur=None):
                    if side_work:
                        side_work.pop(0)()
                        if fill_q:
                            fill_q.popleft()[1]()
                            if fill_q:
                                fill_q.popleft()[1]()
                    elif fill_q:
                        fill_q.popleft()[1]()
                        if fill_q:
                            fill_q.popleft()[1]()

                full_t = [t for t in range(NB_ // 2)
                          if all(cls[n * 4 + k, 2 * t + s] == ZERO
                                 for k in range(4) for s in (0, 1))]
                dr_cov = {i for t in full_t for i in (2 * t, 2 * t + 1)}
                stiles = [i for i in range(NB_) if i not in dr_cov
                          and any(cls[n * 4 + k, i] != SKIP for k in range(4))]
                psoA = ps.tile([128, 512], f32, tag="attno", bufs=4)
                psoB = ps.tile([128, 512], f32, tag="attno", bufs=4)
                qA = qT_sb[0:64, u * T_ + n * 512: u * T_ + n * 512 + 512]
                qB = qT_sb[64:128, u * T_ + n * 512: u * T_ + n * 512 + 512]
                for ti, t in enumerate(full_t):
                    pt8 = sbw.tile([128, 2048], f8, tag="pt8", bufs=4)
                    pt8i = pt8.bitcast(i8)
                    for sub in (0, 1):
                        i = 2 * t + sub
                        pss = ps.tile([128, 1024], f32, tag="big", bufs=2)
                        kA = kT_sb[0:64, u * T_ + i * 128: u * T_ + i * 128 + 128]
                        kB = kT_sb[64:128, u * T_ + i * 128: u * T_ + i * 128 + 128]
                        nc.tensor.matmul(pss[:, 0:512], kA, qA, start=True,
                                         stop=True, skip_group_check=True)
                        nc.tensor.matmul(pss[:, 512:1024], kB, qB, start=True,
                                         stop=True, skip_group_check=True)
                        if USE_SCHRAUDOLPH and exp_cnt[0] % 2 == 0:
                            nc.vector.tensor_scalar(
                                pt8i[:, sub * 1024:(sub + 1) * 1024], pss[:],
                                SCH_MUL, SCH_BIAS, ALU.mult, ALU.add)
                        else:
                            nc.scalar.activation(
                                pt8[:, sub * 1024:(sub + 1) * 1024], pss[:],
                                AF.Exp)
                        exp_cnt[0] += 1
                        slot(pss)
                    last_here = (ti == len(full_t) - 1) and not stiles
                    pt83 = pt8[:].rearrange("p (s x) -> p s x", x=1024)
                    for pso_, h, off in ((psoA, hA, 0), (psoB, hB, 512)):
                        v8v = v8_sb[:, t * (HL * 160) + h * 160:
                                    t * (HL * 160) + (h + 1) * 160] \
                            .rearrange("p (o c) -> p o c", c=80)[:, :, 0:65]
                        nc.tensor.matmul(
                            pso_[0:65, :], v8v, pt83[:, :, off:off + 512],
                            start=(ti == 0), stop=last_here,
                            perf_mode=DR, skip_group_check=True)
                dr_started = bool(full_t)
                last = len(stiles) - 1
                for idx, i in enumerate(stiles):
                    runs = []
                    k = 0
                    while k < 4:
                        k1 = k
                        skipk = cls[n * 4 + k, i] == SKIP
                        while k1 < 4 and (cls[n * 4 + k1, i] == SKIP) == skipk:
                            k1 += 1
                        runs.append((k, k1, skipk))
                        k = k1
                    if runs[0][2]:
                        w0 = runs[0][1] * 128
                        runs = runs[1:]
                    else:
                        w0 = 0
                    pss = ps.tile([128, 1024], f32, tag="big", bufs=2)
                    kA = kT_sb[0:64, u * T_ + i * 128: u * T_ + i * 128 + 128]
                    kB = kT_sb[64:128, u * T_ + i * 128: u * T_ + i * 128 + 128]
                    nc.tensor.matmul(pss[:, w0:512], kA, qA[:, w0:512],
                                     start=True, stop=True,
                                     skip_group_check=True)
                    nc.tensor.matmul(pss[:, 512 + w0:1024], kB, qB[:, w0:512],
                                     start=True, stop=True,
                                     skip_group_check=True)
                    pss3 = pss[:].rearrange("p (o w) -> p o w", w=512)
                    for k in range(4):
                        if cls[n * 4 + k, i] == ADD:
                            mpos = add_pos[(n * 4 + k, i)]
                            mblk = msk_sb[:, mpos * 128:(mpos + 1) * 128]
                            mblk3 = mblk.unsqueeze(1).broadcast_to([128, 2, 128])
                            nc.vector.tensor_add(
                                pss3[:, :, k * 128:(k + 1) * 128],
                                pss3[:, :, k * 128:(k + 1) * 128],
                                mblk3)
                    pt = sbw.tile([128, 1024], bf16, tag="pt", bufs=6)
                    pt3 = pt[:].rearrange("p (o w) -> p o w", w=512)
                    if runs == [(0, 4, False)]:
                        nc.scalar.activation(pt[:], pss[:], AF.Exp)
                    else:
                        for k, k1, skipk in runs:
                            a = max(k * 128, w0)
                            src = pss3[:, :, a: k1 * 128]
                            dst = pt3[:, :, a: k1 * 128]
                            if skipk:
                                nc.gpsimd.memset(dst, 0.0)
                            else:
                                nc.scalar.activation(dst, src, AF.Exp)
                    for k in range(4):
                        if cls[n * 4 + k, i] == ADDBIN:
                            tpos = bin_pos[(n * 4 + k, i)]
                            tblk = tri_sb[:, tpos * 128:(tpos + 1) * 128]
                            tblk3 = tblk.unsqueeze(1).broadcast_to([128, 2, 128])
                            nc.gpsimd.tensor_mul(
                                pt3[:, :, k * 128:(k + 1) * 128],
                                pt3[:, :, k * 128:(k + 1) * 128],
                                tblk3)
                    slot(pss)
                    for pso_, h, off in ((psoA, hA, 0), (psoB, hB, 512)):
                        strip = v_sb[:, i * (HL * 65) + h * 65:
                                     i * (HL * 65) + h * 65 + 65]
                        nc.tensor.matmul(
                            pso_[0:65, w0:512], strip[:, :],
                            pt[:, off + w0:off + 512],
                            start=(idx == 0 and not dr_started),
                            stop=(idx == last),
                            skip_group_check=True)
                while side_work:
                    side_work.pop(0)()
                return psoA, psoB

            def normalize_pair(n, u, psoA, psoB):
                """Denominator rows (PSUM partition 64) copied to SBUF p0
                on ScalarE (reciprocal_approx_fast misreads PSUM base 64),
                one fast approx reciprocal over both heads, partition
                broadcast on Pool, one DVE multiply per head."""
                recd = sbw.tile([1, 1024], f32, tag="recd", bufs=3)
                recf = sbw.tile([1, 1024], f32, tag="recf", bufs=3)
                rb = sbw.tile([64, 1024], f32, tag="rb", bufs=2)
                col = u * T_ + n * 512

                def cA():
                    nc.scalar.copy(recd[0:1, 0:512], psoA[64:65, :])

                def cB():
                    nc.scalar.copy(recd[0:1, 512:1024], psoB[64:65, :])

                def rr():
                    nc.vector.reciprocal_approx_fast(recf[:], recd[:])

                def pb():
                    nc.gpsimd.partition_broadcast(rb[:, :], recf[:, :])

                def mA():
                    nc.vector.tensor_mul(oT_sb[0:64, col:col + 512],
                                         psoA[0:64, :], rb[0:64, 0:512])

                def mB():
                    nc.vector.tensor_mul(oT_sb[64:128, col:col + 512],
                                         psoB[0:64, :], rb[0:64, 512:1024])
                return [cA, cB, rr, pb, mA, mB]

            def out_proj_group(m16, eh):
                pso = ps.tile([128, 512], f32, tag="big", bufs=2)
                for jc in range(J // 128):
                    nc.tensor.matmul(
                        pso[:],
                        oT_sb[:, jc * T_ + m16 * 128: jc * T_ + m16 * 128 + 128],
                        wo_sb[:, jc * E + eh * 512: jc * E + eh * 512 + 512],
                        start=(jc == 0), stop=(jc == J // 128 - 1),
                        skip_group_check=True)
                ob = sbw.tile([128, 512], bf16, tag="ob", bufs=6)
                if (m16 + eh) % 2 == 0:
                    nc.scalar.copy(ob[:], pso[:])
                else:
                    nc.vector.tensor_copy(ob[:], pso[:])
                nc.sync.dma_start(
                    out[m16 * 128:(m16 + 1) * 128,
                        eh * 512:(eh + 1) * 512], ob[:])

            def out_proj_thunks(nn):
                gs = [(m16, eh) for m16 in range(nn * 4, nn * 4 + 4)
                      for eh in range(E // 512)]

                def duo(a, b):
                    def go():
                        out_proj_group(a[0], a[1])
                        out_proj_group(b[0], b[1])
                    return go
                return [duo(gs[i], gs[i + 1]) for i in range(0, len(gs), 2)]

            def out_proj(nn):
                for w in out_proj_thunks(nn):
                    w()

            prevpair = None
            carry = []
            last_op = []
            for pos in range(NMC):
                n = order[pos]
                for u in range(2):
                    work = []
                    if prevpair is not None:
                        ppos, pn, pu, pA, pB = prevpair
                        work = normalize_pair(pn, pu, pA, pB)
                    work += carry
                    carry = []
                    while fill and fill[0][0] <= pos:
                        fill.popleft()[1]()
                    psoA, psoB = s_loop_pair(n, u, work, fill)
                    if prevpair is not None and pu == 1:
                        if ppos == NMC - 2:
                            last_op = out_proj_thunks(pn)
                        else:
                            carry = out_proj_thunks(pn)
                    prevpair = (pos, n, u, psoA, psoB)
            for w in carry:
                w()
            ppos, pn, pu, pA, pB = prevpair
            wAB = normalize_pair(pn, pu, pA, pB)
            lo = last_op if NMC >= 2 else []
            for idx in range(max(len(wAB), len(lo))):
                if idx < len(wAB):
                    wAB[idx]()
                if idx < len(lo):
                    lo[idx]()
            out_proj(pn)

    nc.compile()
    return nc


def _get_program(T_, cls):
    key = (T_, tuple(map(tuple, cls.tolist())))
    if key not in _prog_cache:
        _prog_cache[key] = _build(T_, key[1])
    return _prog_cache[key]


def _numpy_ref(query, attn_mask, key_padding_mask, Wq, bq, Wk, bk, Wv, bv,
               Wo, bo):
    """Exact-semantics fallback (mirrors reference.py in numpy)."""
    q = (query @ Wq.T + bq) * SCALE
    k = query @ Wk.T + bk
    v = query @ Wv.T + bv

    def shp(x):
        return x.reshape(T, B * H, HD).transpose(1, 0, 2)

    q, k, v = shp(q), shp(k), shp(v)
    w = np.einsum('bth,bsh->bts', q, k).reshape(B, H, T, T) + attn_mask
    w = np.where(key_padding_mask[:, None, None, :], -np.inf, w)
    w = w - w.max(axis=-1, keepdims=True)
    ew = np.exp(w)
    p = (ew / ew.sum(axis=-1, keepdims=True)).reshape(B * H, T, T)
    o = np.einsum('bts,bsh->bth', p, v.reshape(B * H, T, HD))
    o = o.transpose(1, 0, 2).reshape(T, B, E)
    return (o @ Wo.T + bo).astype(np.float32)


def _prep_inputs(query, attn_mask, Wq, bq, Wk, Wv, Wo, cls):
    """Build the 8 per-core input maps."""
    bf = np.float16
    f8np = ml_dtypes.float8_e4m3
    add_blocks = [(mb, sb) for mb in range(T // 128) for sb in range(T // 128)
                  if cls[mb, sb] == ADD]
    n_add = len(add_blocks)
    if n_add:
        mskp = np.empty((128, n_add * 128), np.float32)
        for i, (mb, sb) in enumerate(add_blocks):
            blk = attn_mask[mb * 128:(mb + 1) * 128, sb * 128:(sb + 1) * 128]
            mskp[:, i * 128:(i + 1) * 128] = np.ascontiguousarray(blk.T)
    else:
        mskp = np.zeros((128, 128), np.float32)
    bin_blocks = [(mb, sb) for mb in range(T // 128) for sb in range(T // 128)
                  if cls[mb, sb] == ADDBIN]
    if bin_blocks:
        trip = np.empty((128, len(bin_blocks) * 128), bf)
        for i, (mb, sb) in enumerate(bin_blocks):
            blk = attn_mask[mb * 128:(mb + 1) * 128, sb * 128:(sb + 1) * 128]
            trip[:, i * 128:(i + 1) * 128] = (blk.T == 0.0).astype(bf)
    else:
        trip = np.zeros((128, 128), bf)

    in_maps = []
    for core in range(NCORES):
        b = core // (NCORES // B)
        jsl = slice((core % (NCORES // B)) * J, (core % (NCORES // B)) * J + J)
        EC_, J_ = E // 128, J

        x = np.ascontiguousarray(query[:, b, :].T)  # [E, T] f32
        xTb_c = np.ascontiguousarray(
            x.reshape(EC_, 128, NB, 128).transpose(1, 2, 0, 3)
            .reshape(128, NB * EC_ * 128)).astype(bf)
        xT8_c = np.ascontiguousarray(
            x.reshape(EC2, 2, 128, T).transpose(2, 0, 1, 3)
            .reshape(128, EC2 * 2 * T)).astype(f8np)

        def w8pack(W):
            wt = W[jsl, :].T * np.float32(8.0)  # [E, J], x8 prescale
            return np.ascontiguousarray(
                wt.reshape(EC2, 2, 128, 2, 128).transpose(2, 0, 3, 1, 4)
                .reshape(128, EC2 * 2 * 2 * 128)).astype(f8np)

        def sb_layout(wT):  # [E, J] -> SBUF [128, EC*J]
            return np.ascontiguousarray(
                wT.reshape(EC_, 128, J_).transpose(1, 0, 2)
                .reshape(128, EC_ * J_))

        wq_l = sb_layout((Wq[jsl, :] * np.float32(SCALE)).T)
        wk_l = sb_layout(Wk[jsl, :].T)
        wv_l = sb_layout(Wv[jsl, :].T)

        def usplit(wl, u):  # [128, EC*J] -> u-half [128, EC*128]
            return np.ascontiguousarray(
                wl.reshape(128, EC_, 2, 128)[:, :, u, :]
                .reshape(128, EC_ * 128)).astype(bf)

        woT = Wo[:, jsl].T  # [J, E]
        wopack = np.ascontiguousarray(
            woT.reshape(J_ // 128, 128, E).transpose(1, 0, 2)
            .reshape(128, (J_ // 128) * E)).astype(bf)
        bq_c = np.ascontiguousarray(
            (bq[jsl] * np.float32(SCALE)).reshape(2, 128).T)
        in_maps.append({
            "xTb": xTb_c, "xT8": xT8_c,
            "w8q": w8pack(Wq), "w8k": w8pack(Wk),
            "wqpack0": usplit(wq_l, 0), "wqpack1": usplit(wq_l, 1),
            "wkpack0": usplit(wk_l, 0), "wkpack1": usplit(wk_l, 1),
            "wvpack": np.ascontiguousarray(wv_l).astype(bf),
            "wopack": wopack, "bqp": bq_c, "msk": mskp, "tri": trip,
        })
    return in_maps


def _kernel_impl(inputs, trace=False, **run_kwargs):
    query = np.asarray(inputs["query"], np.float32)
    attn_mask = np.asarray(inputs["attn_mask"], np.float32)
    kpm = np.asarray(inputs["key_padding_mask"])
    Wq = np.asarray(inputs["Wq"], np.float32)
    bq = np.asarray(inputs["bq"], np.float32)
    Wk = np.asarray(inputs["Wk"], np.float32)
    bk = np.asarray(inputs["bk"], np.float32)
    Wv = np.asarray(inputs["Wv"], np.float32)
    bv = np.asarray(inputs["bv"], np.float32)
    Wo = np.asarray(inputs["Wo"], np.float32)
    bo = np.asarray(inputs["bo"], np.float32)

    cls = _classify_mask(attn_mask)
    fallback = (
        kpm.any()
        or (attn_mask.max(axis=1) <= NEG_THRESH).any()
        or (cls == ADD).sum() > 24 or (cls == ADDBIN).sum() > 24
        or np.isnan(attn_mask).any()
    )
    if fallback:
        return _numpy_ref(query, attn_mask, kpm, Wq, bq, Wk, bk, Wv, bv,
                          Wo, bo), None

    nc = _get_program(T, cls)
    in_maps = _prep_inputs(query, attn_mask, Wq, bq, Wk, Wv, Wo, cls)
    for attempt in range(3):
        res = run_bass_kernel_spmd(nc, in_maps, core_ids=list(range(NCORES)),
                                   trace=trace, **run_kwargs)
        if all(np.isfinite(r["out"]).all() for r in res.results):
            break
    else:
        return _numpy_ref(query, attn_mask, kpm, Wq, bq, Wk, bk, Wv, bv,
                          Wo, bo), None

    # unshard: sum the 4 row-split partials per batch element (the Wo
    # all-reduce), then add bo and the bv contribution (sum_s p = 1).
    bo_total = bo + Wo @ bv
    out = np.empty((T, B, E), np.float32)
    gsz = NCORES // B
    for b in range(B):
        acc = res.results[b * gsz]["out"].astype(np.float32)
        for c in range(b * gsz + 1, (b + 1) * gsz):
            acc = acc + res.results[c]["out"].astype(np.float32)
        out[:, b, :] = acc + bo_total[None, :]
    return out, res


def kernel(**inputs):
    out, _ = _kernel_impl(inputs, trace=False)
    return out


# revision 21
# speedup vs baseline: 1.3546x; 1.0308x over previous
"""Fused multi-head self-attention (T=2048, B=2, E=1024, H=16) on 8 TRN2 cores.

Sharding: batch*heads across cores — core c handles b = c//4, heads
[(c%4)*4, (c%4)*4+4). Projections are column-split (Wq/Wk/Wv) per core's
heads; Wo is row-split with the cross-core reduction done on the host
during unshard (4 partial [T,E] sums per batch element).

Device kernel (per core, identical SPMD program):
  - q/k projections run as fp8e4 DoubleRow matmuls (K=256 per chunk, 2x
    fewer streams than fp16; weights prescaled x8 on the host; the 1/8
    and softmax scale fold into the PSUM->SBUF copy, split ACT/DVE) into
    fp16 qT/kT; v/out projections stay fp16 for accuracy
  - scores stay fp16 (DoubleRow streams at 1 cycle/output column, so the
    2-head-concurrent fp16 path is already at the PE scores floor); a
    full-E fp16 corner projection overwrites qT/kT cols 0:128 so the
    (0,0) block's tiny-softmax-support rows see fp16-accurate logits
  - exp on full 256-wide tiles alternates between ScalarE (native Exp ->
    fp8) and VectorE (Schraudolph: one fused mult+add fp32->int8 writes
    fp8e4m3 bits of exp(x) directly); diagonal tiles keep accurate ACT
    exp -> fp16 and fp16 AV with fp16 V strips
  - softmax reductions avoided entirely: denominators via a ones-column
    appended to each V strip (row 64 of the AV accumulation), denominator
    rows copied to SBUF p0 on ScalarE, one fast approx reciprocal,
    partition broadcast on GpSimd, one DVE multiply per head
  - causal structure via compile-time block classification: fully-masked
    blocks trimmed out of the streams, binary diagonal blocks masked
    post-exp with 0/1 multiplies on GpSimd, general additive blocks added
    pre-exp on VectorE
  - x arrives twice: fp8 (chunk-pair layout for DR projections) and fp16
    block-major (one 128-row T-block per DMA so v-projections stream
    just-in-time during the first s-loop instead of waiting for 4MB)
  - m-chunks processed [1,3,2,0]; projections/v-groups/corner emitted as
    PE filler between the exp and AV of later s-loop iterations
  - one transient-NaN retry; numpy fallback for exotic masks/key padding
"""
import os
import sys

import numpy as np

for _p in ("/opt/trn_rl_repo", "/root/.axon_site/_ro/trn_rl_repo"):
    if os.path.isdir(_p) and _p not in sys.path:
        sys.path.insert(0, _p)
        break

import ml_dtypes

import concourse.bacc as bacc
import concourse.mybir as mybir
import concourse.tile as tile
from concourse.bass_utils import run_bass_kernel_spmd

f32 = mybir.dt.float32
bf16 = mybir.dt.float16
f8 = mybir.dt.float8e4
i8 = mybir.dt.int8
AF = mybir.ActivationFunctionType
DR = mybir.MatmulPerfMode.DoubleRow
ALU = mybir.AluOpType

T, B, E, H, HD = 2048, 2, 1024, 16, 64
NCORES = 8
HL = (B * H) // NCORES          # heads per core = 4
J = HL * HD                     # per-core projection width = 256
EC = E // 128                   # fp16 e-chunks = 8
EC2 = E // 256                  # fp8 DR e-chunks = 4
NB = T // 128
SCALE = HD ** -0.5
MCH = 512                       # m-chunk width
NEG_THRESH = -1e8               # "fully masked" threshold

SKIP, ZERO, ADD, ADDBIN = 0, 1, 2, 3

# Schraudolph exp -> fp8e4m3 bits: b = 8*log2(e)*x + 8*7 (+0.5 if the
# DVE float->int8 convert truncates instead of rounding)
SCH_MUL = 11.541560327111707
SCH_BIAS = 56.5
USE_SCHRAUDOLPH = False         # fulls exp on DVE (Schraudolph) vs ACT

_prog_cache = {}


def _classify_mask(mask):
    """Classify 128x128 blocks of mask[t_query, s_key]."""
    nb = mask.shape[0] // 128
    blocks = mask.reshape(nb, 128, nb, 128)
    all_skip = (blocks <= NEG_THRESH).all(axis=(1, 3))
    all_zero = (blocks == 0.0).all(axis=(1, 3))
    binary = ((blocks == 0.0) | (blocks <= NEG_THRESH)).all(axis=(1, 3))
    cls = np.where(all_skip, SKIP,
                   np.where(all_zero, ZERO, np.where(binary, ADDBIN, ADD)))
    return cls  # [m_block, s_block]


def _build(T_, cls_key, debug=False):
    cls = np.array(cls_key, dtype=np.int64)
    NB_ = T_ // 128
    NMC = T_ // MCH
    add_blocks = [(mb, sb) for mb in range(NB_) for sb in range(NB_)
                  if cls[mb, sb] == ADD]
    add_pos = {blk: i for i, blk in enumerate(add_blocks)}
    n_add = len(add_blocks)
    bin_blocks = [(mb, sb) for mb in range(NB_) for sb in range(NB_)
                  if cls[mb, sb] == ADDBIN]
    bin_pos = {blk: i for i, blk in enumerate(bin_blocks)}
    n_bin = len(bin_blocks)

    nc = bacc.Bacc("TRN2", target_bir_lowering=False, debug=False)
    xTb = nc.declare_dram_parameter("xTb", [128, NB_ * EC * 128], bf16,
                                    isOutput=False)
    xT8 = nc.declare_dram_parameter("xT8", [128, EC2 * 2 * T_], f8,
                                    isOutput=False)
    w8q = nc.declare_dram_parameter("w8q", [128, EC2 * 512], f8,
                                    isOutput=False)
    w8k = nc.declare_dram_parameter("w8k", [128, EC2 * 512], f8,
                                    isOutput=False)
    wqpack0 = nc.declare_dram_parameter("wqpack0", [128, EC * 128], bf16,
                                        isOutput=False)
    wqpack1 = nc.declare_dram_parameter("wqpack1", [128, EC * 128], bf16,
                                        isOutput=False)
    wkpack0 = nc.declare_dram_parameter("wkpack0", [128, EC * 128], bf16,
                                        isOutput=False)
    wkpack1 = nc.declare_dram_parameter("wkpack1", [128, EC * 128], bf16,
                                        isOutput=False)
    wvpack = nc.declare_dram_parameter("wvpack", [128, EC * J], bf16,
                                       isOutput=False)
    wopack = nc.declare_dram_parameter("wopack", [128, (J // 128) * E], bf16,
                                       isOutput=False)
    bqp = nc.declare_dram_parameter("bqp", [128, 2], f32, isOutput=False)
    msk = nc.declare_dram_parameter("msk", [128, max(n_add, 1) * 128], f32,
                                    isOutput=False)
    tri = nc.declare_dram_parameter("tri", [128, max(n_bin, 1) * 128], bf16,
                                    isOutput=False)
    out = nc.declare_dram_parameter("out", [T_, E], bf16, isOutput=True)

    with tile.TileContext(nc) as tc:
        with nc.allow_low_precision(reason="fp8/fp16 matmuls, fp32 psum"), \
             tc.tile_pool(name="sba", bufs=1) as sba, \
             tc.tile_pool(name="sbw", bufs=1) as sbw, \
             tc.tile_pool(name="ps", bufs=1, space="PSUM") as ps:
            xTb_sb = sba.tile([128, NB_ * EC * 128], bf16)
            xT8_sb = sba.tile([128, EC2 * 2 * T_], f8)
            w8q_sb = sba.tile([128, EC2 * 512], f8)
            w8k_sb = sba.tile([128, EC2 * 512], f8)
            wpack_sb = sba.tile([128, 3 * EC * J], bf16)
            wq_sb = wpack_sb[:, 0:EC * J]
            wk_sb = wpack_sb[:, EC * J:2 * EC * J]
            wv_sb = wpack_sb[:, 2 * EC * J:3 * EC * J]
            wo_sb = sba.tile([128, (J // 128) * E], bf16)
            qT_sb = sba.tile([128, 2 * T_], bf16)
            kT_sb = sba.tile([128, 2 * T_], bf16)
            v_sb = sba.tile([128, HL * NB_ * 65], bf16)
            v8_sb = sba.tile([128, (NB_ // 2) * HL * 160], f8)
            oT_sb = sba.tile([128, 2 * T_], bf16)
            bq_sb = sba.tile([128, 2], f32)
            msk_sb = sba.tile([128, max(n_add, 1) * 128], f32)
            tri_sb = sba.tile([128, max(n_bin, 1) * 128], bf16)

            # ---- input DMAs, ordered for the ramp: fp8 projection data
            # first (first matmul ~3us in), fp16 x block-major so v-groups
            # stream just-in-time during the first s-loop ----
            nc.sync.dma_start(w8k_sb[:], w8k[:, :])
            nc.sync.dma_start(w8q_sb[:], w8q[:, :])
            nc.sync.dma_start(bq_sb[:], bqp[:, :])
            for c in range(EC2):
                nc.sync.dma_start(xT8_sb[:, c * 2 * T_:(c + 1) * 2 * T_],
                                  xT8[:, c * 2 * T_:(c + 1) * 2 * T_])
            for i in range(2):
                nc.sync.dma_start(xTb_sb[:, i * 1024:(i + 1) * 1024],
                                  xTb[:, i * 1024:(i + 1) * 1024])
            nc.sync.dma_start(wv_sb[:, :], wvpack[:, :])
            for i in range(2, NB_):
                nc.sync.dma_start(xTb_sb[:, i * 1024:(i + 1) * 1024],
                                  xTb[:, i * 1024:(i + 1) * 1024])
            v_ones_view = v_sb[:].rearrange("p (x c) -> p x c", c=65)[:, :, 64:65]
            nc.vector.memset(v_ones_view, 1.0)
            v8_ones_view = v8_sb[:].rearrange("p (x c) -> p x c",
                                              c=80)[:, :, 64:65]
            nc.vector.memset(v8_ones_view, 1.0)
            nc.sync.dma_start(wo_sb[:], wopack[:, :])

            def wslc(wsb, u):
                return wsb.rearrange("p (c u x) -> p c u x", u=2,
                                     x=128)[:, :, u, :]
            nc.sync.dma_start(wslc(wq_sb, 0), wqpack0[:, :])
            nc.sync.dma_start(wslc(wq_sb, 1), wqpack1[:, :])
            nc.sync.dma_start(wslc(wk_sb, 0), wkpack0[:, :])
            nc.sync.dma_start(wslc(wk_sb, 1), wkpack1[:, :])
            if n_add:
                nc.sync.dma_start(msk_sb[:], msk[:, :])
            if n_bin:
                nc.sync.dma_start(tri_sb[:], tri[:, :])

            # ---- fp8 DR q/k projection groups -> fp16 qT/kT staging ----
            qk_cnt = [0]

            def qk_group8(nn, u, w8sb, stag, is_q):
                psq = ps.tile([128, 512], f32, tag="big", bufs=2)
                for c in range(EC2):
                    w8v = w8sb[:, c * 512 + u * 256: c * 512 + u * 256 + 256] \
                        .rearrange("p (r m) -> p r m", r=2)
                    x8v = xT8_sb[:, c * 2 * T_:(c + 1) * 2 * T_] \
                        .rearrange("p (r m) -> p r m",
                                   r=2)[:, :, nn * 512:(nn + 1) * 512]
                    nc.tensor.matmul(psq[:], w8v, x8v, start=(c == 0),
                                     stop=(c == EC2 - 1), perf_mode=DR)
                # copies on DVE: ScalarE stays a pure exp conveyor (its
                # latency gates the s-loop pipeline and HAM doesn't
                # throttle it)
                dst = stag[:, u * T_ + nn * 512: u * T_ + nn * 512 + 512]
                if is_q:
                    nc.vector.tensor_scalar(dst, psq[:], SCALE / 8.0,
                                            bq_sb[:, u:u + 1], ALU.mult,
                                            ALU.add)
                else:
                    nc.vector.tensor_scalar_mul(dst, psq[:], 0.125)

            def k_group8(nn, u):
                qk_group8(nn, u, w8k_sb, kT_sb, False)

            def q_group8(nn, u):
                qk_group8(nn, u, w8q_sb, qT_sb, True)

            # fp16 full-E corner projection overwrites qT/kT cols 0:128 so
            # the (0,0) block's short-support rows see fp16 logits
            def corner_group(u):
                psc = ps.tile([128, 512], f32, tag="big", bufs=2)
                for c in range(EC):
                    nc.tensor.matmul(
                        psc[:, 0:128],
                        wq_sb[:, c * J + u * 128: c * J + (u + 1) * 128],
                        xTb_sb[:, c * 128: c * 128 + 128],
                        start=(c == 0), stop=(c == EC - 1))
                for c in range(EC):
                    nc.tensor.matmul(
                        psc[:, 128:256],
                        wk_sb[:, c * J + u * 128: c * J + (u + 1) * 128],
                        xTb_sb[:, c * 128: c * 128 + 128],
                        start=(c == 0), stop=(c == EC - 1))
                nc.vector.tensor_scalar_add(qT_sb[:, u * T_: u * T_ + 128],
                                            psc[:, 0:128], bq_sb[:, u:u + 1])
                nc.vector.tensor_copy(kT_sb[:, u * T_: u * T_ + 128],
                                      psc[:, 128:256])

            def v_group(i):
                psv = ps.tile([128, 512], f32, tag="big", bufs=2)
                for c in range(EC):
                    nc.tensor.matmul(
                        psv[:, 0:J],
                        xTb_sb[:, i * 1024 + c * 128: i * 1024 + c * 128 + 128],
                        wv_sb[:, c * J:(c + 1) * J],
                        start=(c == 0), stop=(c == EC - 1))
                # ones-last strips: [v0..v63, 1] per (block, head) — the
                # denominator lands at PSUM partition 64 (legal AP base)
                dstv = v_sb[:, i * (HL * 65):(i + 1) * (HL * 65)] \
                    .rearrange("p (h c) -> p h c", c=65)[:, :, 0:64]
                srcv = psv[:, 0:J].rearrange("p (h c) -> p h c", c=64)
                nc.vector.tensor_copy(dstv, srcv)
                # fp8 strips straight from PSUM on DVE
                t, par = i // 2, i % 2
                dst8 = v8_sb[:, t * (HL * 160):(t + 1) * (HL * 160)] \
                    .rearrange("p (h c) -> p h c",
                               c=160)[:, :, par * 80: par * 80 + 64]
                nc.vector.tensor_copy(dst8, srcv)

            from collections import deque
            order = [1, 3, 2, 0] if NMC == 4 else list(range(NMC))
            first = order[0]
            for u in range(2):
                for kk in range(first + 1):
                    k_group8(kk, u)
                q_group8(first, u)
            for i in range(2):
                v_group(i)
            vdone = 2

            def _k_thunk(nn, u):
                return lambda: k_group8(nn, u)

            def _q_thunk(nn, u):
                return lambda: q_group8(nn, u)

            def _c_thunk(u):
                return lambda: corner_group(u)

            def _v_thunk(i):
                return lambda: v_group(i)

            fill = deque()
            need0 = min(4 * first + 4, NB_)
            for i in range(vdone, need0):
                fill.append((1, _v_thunk(i)))
            vdone = need0
            maxk = first
            for pos in range(1, NMC):
                nn = order[pos]
                for u in range(2):
                    for kk in range(maxk + 1, nn + 1):
                        fill.append((pos, _k_thunk(kk, u)))
                    fill.append((pos, _q_thunk(nn, u)))
                    if nn == 0:
                        fill.append((pos, _c_thunk(u)))
                maxk = max(maxk, nn)
                need = min(4 * nn + 4, NB_)
                for i in range(vdone, need):
                    fill.append((pos, _v_thunk(i)))
                vdone = max(vdone, need)
            for i in range(vdone, NB_):
                fill.append((NMC - 1, _v_thunk(i)))

            exp_cnt = [0]

            # ---- attention: DR scores (heads on PE halves 0/64), exp
            # split DVE-Schraudolph/ACT, split-K AV, pipelined normalize,
            # deferred out-proj ----
            def s_loop_pair(n, u, side_work=(), fill_q=None):
                side_work = list(side_work)
                hA, hB = 2 * u, 2 * u + 1

                def slot(pss_cur=None):
                    if side_work:
                        side_work.pop(0)()
                        if fill_q:
                            fill_q.popleft()[1]()
                            if fill_q:
                                fill_q.popleft()[1]()
                    elif fill_q:
                        fill_q.popleft()[1]()
                        if fill_q:
                            fill_q.popleft()[1]()

                full_t = [t for t in range(NB_ // 2)
                          if all(cls[n * 4 + k, 2 * t + s] == ZERO
                                 for k in range(4) for s in (0, 1))]
                dr_cov = {i for t in full_t for i in (2 * t, 2 * t + 1)}
                stiles = [i for i in range(NB_) if i not in dr_cov
                          and any(cls[n * 4 + k, i] != SKIP for k in range(4))]
                psoA = ps.tile([128, 512], f32, tag="attno", bufs=4)
                psoB = ps.tile([128, 512], f32, tag="attno", bufs=4)
                qA = qT_sb[0:64, u * T_ + n * 512: u * T_ + n * 512 + 512]
                qB = qT_sb[64:128, u * T_ + n * 512: u * T_ + n * 512 + 512]
                for ti, t in enumerate(full_t):
                    pt8 = sbw.tile([128, 2048], f8, tag="pt8", bufs=4)
                    pt8i = pt8.bitcast(i8)
                    for sub in (0, 1):
                        i = 2 * t + sub
                        pss = ps.tile([128, 1024], f32, tag="big", bufs=2)
                        kA = kT_sb[0:64, u * T_ + i * 128: u * T_ + i * 128 + 128]
                        kB = kT_sb[64:128, u * T_ + i * 128: u * T_ + i * 128 + 128]
                        nc.tensor.matmul(pss[:, 0:512], kA, qA, start=True,
                                         stop=True, skip_group_check=True)
                        nc.tensor.matmul(pss[:, 512:1024], kB, qB, start=True,
                                         stop=True, skip_group_check=True)
                        if USE_SCHRAUDOLPH and exp_cnt[0] % 2 == 0:
                            nc.vector.tensor_scalar(
                                pt8i[:, sub * 1024:(sub + 1) * 1024], pss[:],
                                SCH_MUL, SCH_BIAS, ALU.mult, ALU.add)
                        else:
                            nc.scalar.activation(
                                pt8[:, sub * 1024:(sub + 1) * 1024], pss[:],
                                AF.Exp)
                        exp_cnt[0] += 1
                        slot(pss)
                    last_here = (ti == len(full_t) - 1) and not stiles
                    pt83 = pt8[:].rearrange("p (s x) -> p s x", x=1024)
                    for pso_, h, off in ((psoA, hA, 0), (psoB, hB, 512)):
                        v8v = v8_sb[:, t * (HL * 160) + h * 160:
                                    t * (HL * 160) + (h + 1) * 160] \
                            .rearrange("p (o c) -> p o c", c=80)[:, :, 0:65]
                        nc.tensor.matmul(
                            pso_[0:65, :], v8v, pt83[:, :, off:off + 512],
                            start=(ti == 0), stop=last_here,
                            perf_mode=DR, skip_group_check=True)
                dr_started = bool(full_t)
                last = len(stiles) - 1
                for idx, i in enumerate(stiles):
                    runs = []
                    k = 0
                    while k < 4:
                        k1 = k
                        skipk = cls[n * 4 + k, i] == SKIP
                        while k1 < 4 and (cls[n * 4 + k1, i] == SKIP) == skipk:
                            k1 += 1
                        runs.append((k, k1, skipk))
                        k = k1
                    if runs[0][2]:
                        w0 = runs[0][1] * 128
                        runs = runs[1:]
                    else:
                        w0 = 0
                    pss = ps.tile([128, 1024], f32, tag="big", bufs=2)
                    kA = kT_sb[0:64, u * T_ + i * 128: u * T_ + i * 128 + 128]
                    kB = kT_sb[64:128, u * T_ + i * 128: u * T_ + i * 128 + 128]
                    nc.tensor.matmul(pss[:, w0:512], kA, qA[:, w0:512],
                                     start=True, stop=True,
                                     skip_group_check=True)
                    nc.tensor.matmul(pss[:, 512 + w0:1024], kB, qB[:, w0:512],
                                     start=True, stop=True,
                                     skip_group_check=True)
                    pss3 = pss[:].rearrange("p (o w) -> p o w", w=512)
                    for k in range(4):
                        if cls[n * 4 + k, i] == ADD:
                            mpos = add_pos[(n * 4 + k, i)]
                            mblk = msk_sb[:, mpos * 128:(mpos + 1) * 128]
                            mblk3 = mblk.unsqueeze(1).broadcast_to([128, 2, 128])
                            nc.vector.tensor_add(
                                pss3[:, :, k * 128:(k + 1) * 128],
                                pss3[:, :, k * 128:(k + 1) * 128],
                                mblk3)
                    pt = sbw.tile([128, 1024], bf16, tag="pt", bufs=6)
                    pt3 = pt[:].rearrange("p (o w) -> p o w", w=512)
                    if runs == [(0, 4, False)]:
                        nc.scalar.activation(pt[:], pss[:], AF.Exp)
                    else:
                        for k, k1, skipk in runs:
                            a = max(k * 128, w0)
                            src = pss3[:, :, a: k1 * 128]
                            dst = pt3[:, :, a: k1 * 128]
                            if skipk:
                                nc.gpsimd.memset(dst, 0.0)
                            else:
                                nc.scalar.activation(dst, src, AF.Exp)
                    for k in range(4):
                        if cls[n * 4 + k, i] == ADDBIN:
                            tpos = bin_pos[(n * 4 + k, i)]
                            tblk = tri_sb[:, tpos * 128:(tpos + 1) * 128]
                            tblk3 = tblk.unsqueeze(1).broadcast_to([128, 2, 128])
                            nc.gpsimd.tensor_mul(
                                pt3[:, :, k * 128:(k + 1) * 128],
                                pt3[:, :, k * 128:(k + 1) * 128],
                                tblk3)
                    slot(pss)
                    for pso_, h, off in ((psoA, hA, 0), (psoB, hB, 512)):
                        strip = v_sb[:, i * (HL * 65) + h * 65:
                                     i * (HL * 65) + h * 65 + 65]
                        nc.tensor.matmul(
                            pso_[0:65, w0:512], strip[:, :],
                            pt[:, off + w0:off + 512],
                            start=(idx == 0 and not dr_started),
                            stop=(idx == last),
                            skip_group_check=True)
                while side_work:
                    side_work.pop(0)()
                return psoA, psoB

            def normalize_pair(n, u, psoA, psoB):
                """Denominator rows (PSUM partition 64) copied to SBUF p0
                on ScalarE (reciprocal_approx_fast misreads PSUM base 64),
                one fast approx reciprocal over both heads, partition
                broadcast on Pool, one DVE multiply per head."""
                recd = sbw.tile([1, 1024], f32, tag="recd", bufs=3)
                recf = sbw.tile([1, 1024], f32, tag="recf", bufs=3)
                rb = sbw.tile([64, 1024], f32, tag="rb", bufs=2)
                col = u * T_ + n * 512

                def cA():
                    nc.vector.tensor_copy(recd[0:1, 0:512], psoA[64:65, :])

                def cB():
                    nc.vector.tensor_copy(recd[0:1, 512:1024], psoB[64:65, :])

                def rr():
                    nc.vector.reciprocal_approx_fast(recf[:], recd[:])

                def pb():
                    nc.gpsimd.partition_broadcast(rb[:, :], recf[:, :])

                def mA():
                    nc.vector.tensor_mul(oT_sb[0:64, col:col + 512],
                                         psoA[0:64, :], rb[0:64, 0:512])

                def mB():
                    nc.vector.tensor_mul(oT_sb[64:128, col:col + 512],
                                         psoB[0:64, :], rb[0:64, 512:1024])
                return [cA, cB, rr, pb, mA, mB]

            def out_proj_group(m16, eh):
                pso = ps.tile([128, 512], f32, tag="big", bufs=2)
                for jc in range(J // 128):
                    nc.tensor.matmul(
                        pso[:],
                        oT_sb[:, jc * T_ + m16 * 128: jc * T_ + m16 * 128 + 128],
                        wo_sb[:, jc * E + eh * 512: jc * E + eh * 512 + 512],
                        start=(jc == 0), stop=(jc == J // 128 - 1),
                        skip_group_check=True)
                ob = sbw.tile([128, 512], bf16, tag="ob", bufs=6)
                nc.vector.tensor_copy(ob[:], pso[:])
                nc.sync.dma_start(
                    out[m16 * 128:(m16 + 1) * 128,
                        eh * 512:(eh + 1) * 512], ob[:])

            def out_proj_thunks(nn):
                gs = [(m16, eh) for m16 in range(nn * 4, nn * 4 + 4)
                      for eh in range(E // 512)]

                def duo(a, b):
                    def go():
                        out_proj_group(a[0], a[1])
                        out_proj_group(b[0], b[1])
                    return go
                return [duo(gs[i], gs[i + 1]) for i in range(0, len(gs), 2)]

            def out_proj(nn):
                for w in out_proj_thunks(nn):
                    w()

            prevpair = None
            carry = []
            last_op = []
            for pos in range(NMC):
                n = order[pos]
                for u in range(2):
                    work = []
                    if prevpair is not None:
                        ppos, pn, pu, pA, pB = prevpair
                        work = normalize_pair(pn, pu, pA, pB)
                    work += carry
                    carry = []
                    while fill and fill[0][0] <= pos:
                        fill.popleft()[1]()
                    psoA, psoB = s_loop_pair(n, u, work, fill)
                    if prevpair is not None and pu == 1:
                        if ppos == NMC - 2:
                            last_op = out_proj_thunks(pn)
                        else:
                            carry = out_proj_thunks(pn)
                    prevpair = (pos, n, u, psoA, psoB)
            for w in carry:
                w()
            ppos, pn, pu, pA, pB = prevpair
            wAB = normalize_pair(pn, pu, pA, pB)
            lo = last_op if NMC >= 2 else []
            for idx in range(max(len(wAB), len(lo))):
                if idx < len(wAB):
                    wAB[idx]()
                if idx < len(lo):
                    lo[idx]()
            out_proj(pn)

    nc.compile()
    return nc


def _get_program(T_, cls):
    key = (T_, tuple(map(tuple, cls.tolist())))
    if key not in _prog_cache:
        _prog_cache[key] = _build(T_, key[1])
    return _prog_cache[key]


def _numpy_ref(query, attn_mask, key_padding_mask, Wq, bq, Wk, bk, Wv, bv,
               Wo, bo):
    """Exact-semantics fallback (mirrors reference.py in numpy)."""
    q = (query @ Wq.T + bq) * SCALE
    k = query @ Wk.T + bk
    v = query @ Wv.T + bv

    def shp(x):
        return x.reshape(T, B * H, HD).transpose(1, 0, 2)

    q, k, v = shp(q), shp(k), shp(v)
    w = np.einsum('bth,bsh->bts', q, k).reshape(B, H, T, T) + attn_mask
    w = np.where(key_padding_mask[:, None, None, :], -np.inf, w)
    w = w - w.max(axis=-1, keepdims=True)
    ew = np.exp(w)
    p = (ew / ew.sum(axis=-1, keepdims=True)).reshape(B * H, T, T)
    o = np.einsum('bts,bsh->bth', p, v.reshape(B * H, T, HD))
    o = o.transpose(1, 0, 2).reshape(T, B, E)
    return (o @ Wo.T + bo).astype(np.float32)


def _prep_inputs(query, attn_mask, Wq, bq, Wk, Wv, Wo, cls):
    """Build the 8 per-core input maps."""
    bf = np.float16
    f8np = ml_dtypes.float8_e4m3
    add_blocks = [(mb, sb) for mb in range(T // 128) for sb in range(T // 128)
                  if cls[mb, sb] == ADD]
    n_add = len(add_blocks)
    if n_add:
        mskp = np.empty((128, n_add * 128), np.float32)
        for i, (mb, sb) in enumerate(add_blocks):
            blk = attn_mask[mb * 128:(mb + 1) * 128, sb * 128:(sb + 1) * 128]
            mskp[:, i * 128:(i + 1) * 128] = np.ascontiguousarray(blk.T)
    else:
        mskp = np.zeros((128, 128), np.float32)
    bin_blocks = [(mb, sb) for mb in range(T // 128) for sb in range(T // 128)
                  if cls[mb, sb] == ADDBIN]
    if bin_blocks:
        trip = np.empty((128, len(bin_blocks) * 128), bf)
        for i, (mb, sb) in enumerate(bin_blocks):
            blk = attn_mask[mb * 128:(mb + 1) * 128, sb * 128:(sb + 1) * 128]
            trip[:, i * 128:(i + 1) * 128] = (blk.T == 0.0).astype(bf)
    else:
        trip = np.zeros((128, 128), bf)

    in_maps = []
    for core in range(NCORES):
        b = core // (NCORES // B)
        jsl = slice((core % (NCORES // B)) * J, (core % (NCORES // B)) * J + J)
        EC_, J_ = E // 128, J

        x = np.ascontiguousarray(query[:, b, :].T)  # [E, T] f32
        xTb_c = np.ascontiguousarray(
            x.reshape(EC_, 128, NB, 128).transpose(1, 2, 0, 3)
            .reshape(128, NB * EC_ * 128)).astype(bf)
        xT8_c = np.ascontiguousarray(
            x.reshape(EC2, 2, 128, T).transpose(2, 0, 1, 3)
            .reshape(128, EC2 * 2 * T)).astype(f8np)

        def w8pack(W):
            wt = W[jsl, :].T * np.float32(8.0)  # [E, J], x8 prescale
            return np.ascontiguousarray(
                wt.reshape(EC2, 2, 128, 2, 128).transpose(2, 0, 3, 1, 4)
                .reshape(128, EC2 * 2 * 2 * 128)).astype(f8np)

        def sb_layout(wT):  # [E, J] -> SBUF [128, EC*J]
            return np.ascontiguousarray(
                wT.reshape(EC_, 128, J_).transpose(1, 0, 2)
                .reshape(128, EC_ * J_))

        wq_l = sb_layout((Wq[jsl, :] * np.float32(SCALE)).T)
        wk_l = sb_layout(Wk[jsl, :].T)
        wv_l = sb_layout(Wv[jsl, :].T)

        def usplit(wl, u):  # [128, EC*J] -> u-half [128, EC*128]
            return np.ascontiguousarray(
                wl.reshape(128, EC_, 2, 128)[:, :, u, :]
                .reshape(128, EC_ * 128)).astype(bf)

        woT = Wo[:, jsl].T  # [J, E]
        wopack = np.ascontiguousarray(
            woT.reshape(J_ // 128, 128, E).transpose(1, 0, 2)
            .reshape(128, (J_ // 128) * E)).astype(bf)
        bq_c = np.ascontiguousarray(
            (bq[jsl] * np.float32(SCALE)).reshape(2, 128).T)
        in_maps.append({
            "xTb": xTb_c, "xT8": xT8_c,
            "w8q": w8pack(Wq), "w8k": w8pack(Wk),
            "wqpack0": usplit(wq_l, 0), "wqpack1": usplit(wq_l, 1),
            "wkpack0": usplit(wk_l, 0), "wkpack1": usplit(wk_l, 1),
            "wvpack": np.ascontiguousarray(wv_l).astype(bf),
            "wopack": wopack, "bqp": bq_c, "msk": mskp, "tri": trip,
        })
    return in_maps


def _kernel_impl(inputs, trace=False, **run_kwargs):
    query = np.asarray(inputs["query"], np.float32)
    attn_mask = np.asarray(inputs["attn_mask"], np.float32)
    kpm = np.asarray(inputs["key_padding_mask"])
    Wq = np.asarray(inputs["Wq"], np.float32)
    bq = np.asarray(inputs["bq"], np.float32)
    Wk = np.asarray(inputs["Wk"], np.float32)
    bk = np.asarray(inputs["bk"], np.float32)
    Wv = np.asarray(inputs["Wv"], np.float32)
    bv = np.asarray(inputs["bv"], np.float32)
    Wo = np.asarray(inputs["Wo"], np.float32)
    bo = np.asarray(inputs["bo"], np.float32)

    cls = _classify_mask(attn_mask)
    fallback = (
        kpm.any()
        or (attn_mask.max(axis=1) <= NEG_THRESH).any()
        or (cls == ADD).sum() > 24 or (cls == ADDBIN).sum() > 24
        or np.isnan(attn_mask).any()
    )
    if fallback:
        return _numpy_ref(query, attn_mask, kpm, Wq, bq, Wk, bk, Wv, bv,
                          Wo, bo), None

    nc = _get_program(T, cls)
    in_maps = _prep_inputs(query, attn_mask, Wq, bq, Wk, Wv, Wo, cls)
    for attempt in range(3):
        res = run_bass_kernel_spmd(nc, in_maps, core_ids=list(range(NCORES)),
                                   trace=trace, **run_kwargs)
        if all(np.isfinite(r["out"]).all() for r in res.results):
            break
    else:
        return _numpy_ref(query, attn_mask, kpm, Wq, bq, Wk, bk, Wv, bv,
                          Wo, bo), None

    # unshard: sum the 4 row-split partials per batch element (the Wo
    # all-reduce), then add bo and the bv contribution (sum_s p = 1).
    bo_total = bo + Wo @ bv
    out = np.empty((T, B, E), np.float32)
    gsz = NCORES // B
    for b in range(B):
        acc = res.results[b * gsz]["out"].astype(np.float32)
        for c in range(b * gsz + 1, (b + 1) * gsz):
            acc = acc + res.results[c]["out"].astype(np.float32)
        out[:, b, :] = acc + bo_total[None, :]
    return out, res


def kernel(**inputs):
    out, _ = _kernel_impl(inputs, trace=False)
    return out


# revision 24
# speedup vs baseline: 1.6733x; 1.2352x over previous
"""Fused multi-head self-attention (T=2048, B=2, E=1024, H=16) on 8 TRN2 cores.

Sharding: batch*heads across cores — core c handles b = c//4, heads
[(c%4)*4, (c%4)*4+4). Projections are column-split (Wq/Wk/Wv) per core's
heads; Wo is row-split with the cross-core reduction done on the host
during unshard (4 partial [T,E] sums per batch element).

Device kernel (per core, identical SPMD program):
  - q/k projections run as fp8e4 DoubleRow matmuls (K=256 per chunk, 2x
    fewer streams than fp16; weights prescaled x8 on the host; the 1/8
    and softmax scale fold into the PSUM->SBUF copy, split ACT/DVE) into
    fp16 qT/kT; v/out projections stay fp16 for accuracy
  - scores stay fp16 (DoubleRow streams at 1 cycle/output column, so the
    2-head-concurrent fp16 path is already at the PE scores floor); a
    full-E fp16 corner projection overwrites qT/kT cols 0:128 so the
    (0,0) block's tiny-softmax-support rows see fp16-accurate logits
  - exp on full 256-wide tiles alternates between ScalarE (native Exp ->
    fp8) and VectorE (Schraudolph: one fused mult+add fp32->int8 writes
    fp8e4m3 bits of exp(x) directly); diagonal tiles keep accurate ACT
    exp -> fp16 and fp16 AV with fp16 V strips
  - softmax reductions avoided entirely: denominators via a ones-column
    appended to each V strip (row 64 of the AV accumulation), denominator
    rows copied to SBUF p0 on ScalarE, one fast approx reciprocal,
    partition broadcast on GpSimd, one DVE multiply per head
  - causal structure via compile-time block classification: fully-masked
    blocks trimmed out of the streams, binary diagonal blocks masked
    post-exp with 0/1 multiplies on GpSimd, general additive blocks added
    pre-exp on VectorE
  - x arrives twice: fp8 (chunk-pair layout for DR projections) and fp16
    block-major (one 128-row T-block per DMA so v-projections stream
    just-in-time during the first s-loop instead of waiting for 4MB)
  - m-chunks processed [1,3,2,0]; projections/v-groups/corner emitted as
    PE filler between the exp and AV of later s-loop iterations
  - one transient-NaN retry; numpy fallback for exotic masks/key padding
"""
import os
import sys

import numpy as np

for _p in ("/opt/trn_rl_repo", "/root/.axon_site/_ro/trn_rl_repo"):
    if os.path.isdir(_p) and _p not in sys.path:
        sys.path.insert(0, _p)
        break

import ml_dtypes

import concourse.bacc as bacc
import concourse.mybir as mybir
import concourse.tile as tile
from concourse.bass_utils import run_bass_kernel_spmd

f32 = mybir.dt.float32
bf16 = mybir.dt.float16
f8 = mybir.dt.float8e4
i8 = mybir.dt.int8
AF = mybir.ActivationFunctionType
DR = mybir.MatmulPerfMode.DoubleRow
ALU = mybir.AluOpType

T, B, E, H, HD = 2048, 2, 1024, 16, 64
NCORES = 8
HL = (B * H) // NCORES          # heads per core = 4
J = HL * HD                     # per-core projection width = 256
EC = E // 128                   # fp16 e-chunks = 8
EC2 = E // 256                  # fp8 DR e-chunks = 4
NB = T // 128
SCALE = HD ** -0.5
MCH = 512                       # m-chunk width
NEG_THRESH = -1e8               # "fully masked" threshold

SKIP, ZERO, ADD, ADDBIN = 0, 1, 2, 3

# Schraudolph exp -> fp8e4m3 bits: b = 8*log2(e)*x + 8*7 (+0.5 if the
# DVE float->int8 convert truncates instead of rounding)
SCH_MUL = 11.541560327111707
SCH_BIAS = 56.5
USE_SCHRAUDOLPH = False         # fulls exp on DVE (Schraudolph) vs ACT

_prog_cache = {}


def _classify_mask(mask):
    """Classify 128x128 blocks of mask[t_query, s_key]."""
    nb = mask.shape[0] // 128
    blocks = mask.reshape(nb, 128, nb, 128)
    all_skip = (blocks <= NEG_THRESH).all(axis=(1, 3))
    all_zero = (blocks == 0.0).all(axis=(1, 3))
    binary = ((blocks == 0.0) | (blocks <= NEG_THRESH)).all(axis=(1, 3))
    cls = np.where(all_skip, SKIP,
                   np.where(all_zero, ZERO, np.where(binary, ADDBIN, ADD)))
    return cls  # [m_block, s_block]


def _build(T_, cls_key, debug=False):
    cls = np.array(cls_key, dtype=np.int64)
    NB_ = T_ // 128
    NMC = T_ // MCH
    add_blocks = [(mb, sb) for mb in range(NB_) for sb in range(NB_)
                  if cls[mb, sb] == ADD]
    add_pos = {blk: i for i, blk in enumerate(add_blocks)}
    n_add = len(add_blocks)
    bin_blocks = [(mb, sb) for mb in range(NB_) for sb in range(NB_)
                  if cls[mb, sb] == ADDBIN]
    bin_pos = {blk: i for i, blk in enumerate(bin_blocks)}
    n_bin = len(bin_blocks)

    nc = bacc.Bacc("TRN2", target_bir_lowering=False, debug=False)
    xTb = nc.declare_dram_parameter("xTb", [128, NB_ * EC * 128], bf16,
                                    isOutput=False)
    xT8 = nc.declare_dram_parameter("xT8", [128, EC2 * 2 * T_], f8,
                                    isOutput=False)
    w8q = nc.declare_dram_parameter("w8q", [128, EC2 * 512], f8,
                                    isOutput=False)
    w8k = nc.declare_dram_parameter("w8k", [128, EC2 * 512], f8,
                                    isOutput=False)
    wqpack0 = nc.declare_dram_parameter("wqpack0", [128, EC * 128], bf16,
                                        isOutput=False)
    wqpack1 = nc.declare_dram_parameter("wqpack1", [128, EC * 128], bf16,
                                        isOutput=False)
    wkpack0 = nc.declare_dram_parameter("wkpack0", [128, EC * 128], bf16,
                                        isOutput=False)
    wkpack1 = nc.declare_dram_parameter("wkpack1", [128, EC * 128], bf16,
                                        isOutput=False)
    wvpack = nc.declare_dram_parameter("wvpack", [128, EC * J], bf16,
                                       isOutput=False)
    wopack = nc.declare_dram_parameter("wopack", [128, (J // 128) * E], bf16,
                                       isOutput=False)
    bqp = nc.declare_dram_parameter("bqp", [128, 2], f32, isOutput=False)
    msk = nc.declare_dram_parameter("msk", [128, max(n_add, 1) * 128], f32,
                                    isOutput=False)
    tri = nc.declare_dram_parameter("tri", [128, max(n_bin, 1) * 128], bf16,
                                    isOutput=False)
    out = nc.declare_dram_parameter("out", [T_, E], bf16, isOutput=True)

    with tile.TileContext(nc) as tc:
        with nc.allow_low_precision(reason="fp8/fp16 matmuls, fp32 psum"), \
             tc.tile_pool(name="sba", bufs=1) as sba, \
             tc.tile_pool(name="sbw", bufs=1) as sbw, \
             tc.tile_pool(name="ps", bufs=1, space="PSUM") as ps:
            xTb_sb = sba.tile([128, NB_ * EC * 128], bf16)
            xT8_sb = sba.tile([128, EC2 * 2 * T_], f8)
            w8q_sb = sba.tile([128, EC2 * 512], f8)
            w8k_sb = sba.tile([128, EC2 * 512], f8)
            wpack_sb = sba.tile([128, 3 * EC * J], bf16)
            wq_sb = wpack_sb[:, 0:EC * J]
            wk_sb = wpack_sb[:, EC * J:2 * EC * J]
            wv_sb = wpack_sb[:, 2 * EC * J:3 * EC * J]
            wo_sb = sba.tile([128, (J // 128) * E], bf16)
            qT_sb = sba.tile([128, 2 * T_], bf16)
            kT_sb = sba.tile([128, 2 * T_], bf16)
            v_sb = sba.tile([128, HL * NB_ * 65], bf16)
            v8_sb = sba.tile([128, (NB_ // 2) * HL * 160], f8)
            oT_sb = sba.tile([128, 2 * T_], bf16)
            bq_sb = sba.tile([128, 2], f32)
            msk_sb = sba.tile([128, max(n_add, 1) * 128], f32)
            tri_sb = sba.tile([128, max(n_bin, 1) * 128], bf16)

            # ---- input DMAs, ordered for the ramp: fp8 projection data
            # first (first matmul ~3us in), fp16 x block-major so v-groups
            # stream just-in-time during the first s-loop ----
            nc.sync.dma_start(w8k_sb[:], w8k[:, :])
            nc.sync.dma_start(w8q_sb[:], w8q[:, :])
            nc.sync.dma_start(bq_sb[:], bqp[:, :])
            for c in range(EC2):
                nc.sync.dma_start(xT8_sb[:, c * 2 * T_:(c + 1) * 2 * T_],
                                  xT8[:, c * 2 * T_:(c + 1) * 2 * T_])
            for i in range(2):
                nc.sync.dma_start(xTb_sb[:, i * 1024:(i + 1) * 1024],
                                  xTb[:, i * 1024:(i + 1) * 1024])
            nc.sync.dma_start(wv_sb[:, :], wvpack[:, :])
            for i in range(2, NB_):
                nc.sync.dma_start(xTb_sb[:, i * 1024:(i + 1) * 1024],
                                  xTb[:, i * 1024:(i + 1) * 1024])
            v_ones_view = v_sb[:].rearrange("p (x c) -> p x c", c=65)[:, :, 64:65]
            nc.vector.memset(v_ones_view, 1.0)
            v8_ones_view = v8_sb[:].rearrange("p (x c) -> p x c",
                                              c=80)[:, :, 64:65]
            nc.vector.memset(v8_ones_view, 1.0)
            nc.sync.dma_start(wo_sb[:], wopack[:, :])

            def wslc(wsb, u):
                return wsb.rearrange("p (c u x) -> p c u x", u=2,
                                     x=128)[:, :, u, :]
            nc.sync.dma_start(wslc(wq_sb, 0), wqpack0[:, :])
            nc.sync.dma_start(wslc(wq_sb, 1), wqpack1[:, :])
            nc.sync.dma_start(wslc(wk_sb, 0), wkpack0[:, :])
            nc.sync.dma_start(wslc(wk_sb, 1), wkpack1[:, :])
            if n_add:
                nc.sync.dma_start(msk_sb[:], msk[:, :])
            if n_bin:
                nc.sync.dma_start(tri_sb[:], tri[:, :])

            # ---- fp8 DR q/k projection groups -> fp16 qT/kT staging ----
            qk_cnt = [0]

            def qk_group8(nn, u, w8sb, stag, is_q):
                psq = ps.tile([128, 512], f32, tag="big", bufs=2)
                for c in range(EC2):
                    w8v = w8sb[:, c * 512 + u * 256: c * 512 + u * 256 + 256] \
                        .rearrange("p (r m) -> p r m", r=2)
                    x8v = xT8_sb[:, c * 2 * T_:(c + 1) * 2 * T_] \
                        .rearrange("p (r m) -> p r m",
                                   r=2)[:, :, nn * 512:(nn + 1) * 512]
                    nc.tensor.matmul(psq[:], w8v, x8v, start=(c == 0),
                                     stop=(c == EC2 - 1), perf_mode=DR)
                # copies on DVE: ScalarE stays a pure exp conveyor (its
                # latency gates the s-loop pipeline and HAM doesn't
                # throttle it)
                dst = stag[:, u * T_ + nn * 512: u * T_ + nn * 512 + 512]
                if is_q:
                    nc.vector.tensor_scalar(dst, psq[:], SCALE / 8.0,
                                            bq_sb[:, u:u + 1], ALU.mult,
                                            ALU.add)
                else:
                    nc.vector.tensor_scalar_mul(dst, psq[:], 0.125)

            def k_group8(nn, u):
                qk_group8(nn, u, w8k_sb, kT_sb, False)

            def q_group8(nn, u):
                qk_group8(nn, u, w8q_sb, qT_sb, True)

            # fp16 full-E corner projection overwrites qT/kT cols 0:128 so
            # the (0,0) block's short-support rows see fp16 logits
            def corner_group(u):
                psc = ps.tile([128, 512], f32, tag="big", bufs=2)
                for c in range(EC):
                    nc.tensor.matmul(
                        psc[:, 0:128],
                        wq_sb[:, c * J + u * 128: c * J + (u + 1) * 128],
                        xTb_sb[:, c * 128: c * 128 + 128],
                        start=(c == 0), stop=(c == EC - 1))
                for c in range(EC):
                    nc.tensor.matmul(
                        psc[:, 128:256],
                        wk_sb[:, c * J + u * 128: c * J + (u + 1) * 128],
                        xTb_sb[:, c * 128: c * 128 + 128],
                        start=(c == 0), stop=(c == EC - 1))
                nc.vector.tensor_scalar_add(qT_sb[:, u * T_: u * T_ + 128],
                                            psc[:, 0:128], bq_sb[:, u:u + 1])
                nc.vector.tensor_copy(kT_sb[:, u * T_: u * T_ + 128],
                                      psc[:, 128:256])

            def v_group(i):
                psv = ps.tile([128, 512], f32, tag="big", bufs=2)
                for c in range(EC):
                    nc.tensor.matmul(
                        psv[:, 0:J],
                        xTb_sb[:, i * 1024 + c * 128: i * 1024 + c * 128 + 128],
                        wv_sb[:, c * J:(c + 1) * J],
                        start=(c == 0), stop=(c == EC - 1))
                # ones-last strips: [v0..v63, 1] per (block, head) — the
                # denominator lands at PSUM partition 64 (legal AP base)
                dstv = v_sb[:, i * (HL * 65):(i + 1) * (HL * 65)] \
                    .rearrange("p (h c) -> p h c", c=65)[:, :, 0:64]
                srcv = psv[:, 0:J].rearrange("p (h c) -> p h c", c=64)
                nc.vector.tensor_copy(dstv, srcv)
                # fp8 strips straight from PSUM on DVE
                t, par = i // 2, i % 2
                dst8 = v8_sb[:, t * (HL * 160):(t + 1) * (HL * 160)] \
                    .rearrange("p (h c) -> p h c",
                               c=160)[:, :, par * 80: par * 80 + 64]
                nc.vector.tensor_copy(dst8, srcv)

            from collections import deque
            order = [1, 3, 2, 0] if NMC == 4 else list(range(NMC))
            first = order[0]
            for u in range(2):
                for kk in range(first + 1):
                    k_group8(kk, u)
                q_group8(first, u)
            for i in range(2):
                v_group(i)
            vdone = 2

            def _k_thunk(nn, u):
                return lambda: k_group8(nn, u)

            def _q_thunk(nn, u):
                return lambda: q_group8(nn, u)

            def _c_thunk(u):
                return lambda: corner_group(u)

            def _v_thunk(i):
                return lambda: v_group(i)

            fill = deque()
            need0 = min(4 * first + 4, NB_)
            for i in range(vdone, need0):
                fill.append((1, _v_thunk(i)))
            vdone = need0
            maxk = first
            for pos in range(1, NMC):
                nn = order[pos]
                for u in range(2):
                    for kk in range(maxk + 1, nn + 1):
                        fill.append((pos, _k_thunk(kk, u)))
                    fill.append((pos, _q_thunk(nn, u)))
                    if nn == 0:
                        fill.append((pos, _c_thunk(u)))
                maxk = max(maxk, nn)
                need = min(4 * nn + 4, NB_)
                for i in range(vdone, need):
                    fill.append((pos, _v_thunk(i)))
                vdone = max(vdone, need)
            for i in range(vdone, NB_):
                fill.append((NMC - 1, _v_thunk(i)))

            exp_cnt = [0]

            # ---- attention: DR scores (heads on PE halves 0/64), exp
            # split DVE-Schraudolph/ACT, split-K AV, pipelined normalize,
            # deferred out-proj ----
            def s_loop_pair(n, u, side_work=(), fill_q=None, prev_stiles=()):
                """Emit this pair's full-tile section; the previous pair's
                deferred stile blocks interleave into it so their long
                exp->mask->AV chains hide behind this pair's PE streams.
                This pair's own stile blocks are returned as thunks."""
                side_work = list(side_work)
                prev_stiles = list(prev_stiles)
                hA, hB = 2 * u, 2 * u + 1

                def slot(pss_cur=None):
                    # priority: prev pair's deferred stiles (their normalize
                    # sits in side_work and depends on them), then normalize/
                    # out-proj side work, then projection filler
                    if prev_stiles:
                        prev_stiles.pop(0)()
                        if fill_q:
                            fill_q.popleft()[1]()
                    elif side_work:
                        side_work.pop(0)()
                        if fill_q:
                            fill_q.popleft()[1]()
                            if fill_q:
                                fill_q.popleft()[1]()
                    elif fill_q:
                        fill_q.popleft()[1]()
                        if fill_q:
                            fill_q.popleft()[1]()

                full_t = [t for t in range(NB_ // 2)
                          if all(cls[n * 4 + k, 2 * t + s] == ZERO
                                 for k in range(4) for s in (0, 1))]
                dr_cov = {i for t in full_t for i in (2 * t, 2 * t + 1)}
                stiles = [i for i in range(NB_) if i not in dr_cov
                          and any(cls[n * 4 + k, i] != SKIP for k in range(4))]
                psoA = ps.tile([128, 512], f32, tag="attno", bufs=4)
                psoB = ps.tile([128, 512], f32, tag="attno", bufs=4)
                qA = qT_sb[0:64, u * T_ + n * 512: u * T_ + n * 512 + 512]
                qB = qT_sb[64:128, u * T_ + n * 512: u * T_ + n * 512 + 512]
                for ti, t in enumerate(full_t):
                    pt8 = sbw.tile([128, 2048], f8, tag="pt8", bufs=4)
                    pt8i = pt8.bitcast(i8)
                    for sub in (0, 1):
                        i = 2 * t + sub
                        pss = ps.tile([128, 1024], f32, tag="big", bufs=2)
                        kA = kT_sb[0:64, u * T_ + i * 128: u * T_ + i * 128 + 128]
                        kB = kT_sb[64:128, u * T_ + i * 128: u * T_ + i * 128 + 128]
                        nc.tensor.matmul(pss[:, 0:512], kA, qA, start=True,
                                         stop=True, skip_group_check=True)
                        nc.tensor.matmul(pss[:, 512:1024], kB, qB, start=True,
                                         stop=True, skip_group_check=True)
                        if USE_SCHRAUDOLPH and exp_cnt[0] % 2 == 0:
                            nc.vector.tensor_scalar(
                                pt8i[:, sub * 1024:(sub + 1) * 1024], pss[:],
                                SCH_MUL, SCH_BIAS, ALU.mult, ALU.add)
                        else:
                            nc.scalar.activation(
                                pt8[:, sub * 1024:(sub + 1) * 1024], pss[:],
                                AF.Exp)
                        exp_cnt[0] += 1
                        slot(pss)
                    last_here = (ti == len(full_t) - 1) and not stiles
                    pt83 = pt8[:].rearrange("p (s x) -> p s x", x=1024)
                    for pso_, h, off in ((psoA, hA, 0), (psoB, hB, 512)):
                        v8v = v8_sb[:, t * (HL * 160) + h * 160:
                                    t * (HL * 160) + (h + 1) * 160] \
                            .rearrange("p (o c) -> p o c", c=80)[:, :, 0:65]
                        nc.tensor.matmul(
                            pso_[0:65, :], v8v, pt83[:, :, off:off + 512],
                            start=(ti == 0), stop=last_here,
                            perf_mode=DR, skip_group_check=True)
                dr_started = bool(full_t)
                last = len(stiles) - 1

                def _stile(idx, i):
                    def go():
                        runs = []
                        k = 0
                        while k < 4:
                            k1 = k
                            skipk = cls[n * 4 + k, i] == SKIP
                            while k1 < 4 and \
                                    (cls[n * 4 + k1, i] == SKIP) == skipk:
                                k1 += 1
                            runs.append((k, k1, skipk))
                            k = k1
                        if runs[0][2]:
                            w0 = runs[0][1] * 128
                            del runs[0]
                        else:
                            w0 = 0
                        pss = ps.tile([128, 1024], f32, tag="big", bufs=2)
                        kA = kT_sb[0:64,
                                   u * T_ + i * 128: u * T_ + i * 128 + 128]
                        kB = kT_sb[64:128,
                                   u * T_ + i * 128: u * T_ + i * 128 + 128]
                        nc.tensor.matmul(pss[:, w0:512], kA, qA[:, w0:512],
                                         start=True, stop=True,
                                         skip_group_check=True)
                        nc.tensor.matmul(pss[:, 512 + w0:1024], kB,
                                         qB[:, w0:512], start=True, stop=True,
                                         skip_group_check=True)
                        pss3 = pss[:].rearrange("p (o w) -> p o w", w=512)
                        for k in range(4):
                            if cls[n * 4 + k, i] == ADD:
                                mpos = add_pos[(n * 4 + k, i)]
                                mblk = msk_sb[:, mpos * 128:(mpos + 1) * 128]
                                mblk3 = mblk.unsqueeze(1) \
                                    .broadcast_to([128, 2, 128])
                                nc.vector.tensor_add(
                                    pss3[:, :, k * 128:(k + 1) * 128],
                                    pss3[:, :, k * 128:(k + 1) * 128],
                                    mblk3)
                        pt = sbw.tile([128, 1024], bf16, tag="pt", bufs=6)
                        pt3 = pt[:].rearrange("p (o w) -> p o w", w=512)
                        if runs == [(0, 4, False)]:
                            nc.scalar.activation(pt[:], pss[:], AF.Exp)
                        else:
                            for k, k1, skipk in runs:
                                a = max(k * 128, w0)
                                src = pss3[:, :, a: k1 * 128]
                                dst = pt3[:, :, a: k1 * 128]
                                if skipk:
                                    nc.gpsimd.memset(dst, 0.0)
                                else:
                                    nc.scalar.activation(dst, src, AF.Exp)
                        # binary diagonal mask post-exp on DVE (16-bit SBUF
                        # operands hit the 2x path, keeping the chain short)
                        for k in range(4):
                            if cls[n * 4 + k, i] == ADDBIN:
                                tpos = bin_pos[(n * 4 + k, i)]
                                tblk = tri_sb[:, tpos * 128:(tpos + 1) * 128]
                                tblk3 = tblk.unsqueeze(1) \
                                    .broadcast_to([128, 2, 128])
                                nc.vector.tensor_mul(
                                    pt3[:, :, k * 128:(k + 1) * 128],
                                    pt3[:, :, k * 128:(k + 1) * 128],
                                    tblk3)
                        for pso_, h, off in ((psoA, hA, 0), (psoB, hB, 512)):
                            strip = v_sb[:, i * (HL * 65) + h * 65:
                                         i * (HL * 65) + h * 65 + 65]
                            nc.tensor.matmul(
                                pso_[0:65, w0:512], strip[:, :],
                                pt[:, off + w0:off + 512],
                                start=(idx == 0 and not dr_started),
                                stop=(idx == last),
                                skip_group_check=True)
                    return go

                stile_thunks = [_stile(idx, i) for idx, i in enumerate(stiles)]
                while prev_stiles:
                    prev_stiles.pop(0)()
                    if fill_q:
                        fill_q.popleft()[1]()
                while side_work:
                    side_work.pop(0)()
                return psoA, psoB, stile_thunks

            def normalize_pair(n, u, psoA, psoB):
                """Denominator rows (PSUM partition 64) copied to SBUF p0
                on ScalarE (reciprocal_approx_fast misreads PSUM base 64),
                one fast approx reciprocal over both heads, partition
                broadcast on Pool, one DVE multiply per head."""
                recd = sbw.tile([1, 1024], f32, tag="recd", bufs=3)
                recf = sbw.tile([1, 1024], f32, tag="recf", bufs=3)
                rb = sbw.tile([64, 1024], f32, tag="rb", bufs=2)
                col = u * T_ + n * 512

                def cA():
                    nc.vector.tensor_copy(recd[0:1, 0:512], psoA[64:65, :])

                def cB():
                    nc.vector.tensor_copy(recd[0:1, 512:1024], psoB[64:65, :])

                def rr():
                    nc.vector.reciprocal_approx_fast(recf[:], recd[:])

                def pb():
                    nc.gpsimd.partition_broadcast(rb[:, :], recf[:, :])

                def mA():
                    nc.vector.tensor_mul(oT_sb[0:64, col:col + 512],
                                         psoA[0:64, :], rb[0:64, 0:512])

                def mB():
                    nc.vector.tensor_mul(oT_sb[64:128, col:col + 512],
                                         psoB[0:64, :], rb[0:64, 512:1024])
                return [cA, cB, rr, pb, mA, mB]

            def out_proj_group(m16, eh):
                pso = ps.tile([128, 512], f32, tag="big", bufs=2)
                for jc in range(J // 128):
                    nc.tensor.matmul(
                        pso[:],
                        oT_sb[:, jc * T_ + m16 * 128: jc * T_ + m16 * 128 + 128],
                        wo_sb[:, jc * E + eh * 512: jc * E + eh * 512 + 512],
                        start=(jc == 0), stop=(jc == J // 128 - 1),
                        skip_group_check=True)
                ob = sbw.tile([128, 512], bf16, tag="ob", bufs=6)
                nc.vector.tensor_copy(ob[:], pso[:])
                nc.sync.dma_start(
                    out[m16 * 128:(m16 + 1) * 128,
                        eh * 512:(eh + 1) * 512], ob[:])

            def out_proj_thunks(nn):
                gs = [(m16, eh) for m16 in range(nn * 4, nn * 4 + 4)
                      for eh in range(E // 512)]

                def duo(a, b):
                    def go():
                        out_proj_group(a[0], a[1])
                        out_proj_group(b[0], b[1])
                    return go
                return [duo(gs[i], gs[i + 1]) for i in range(0, len(gs), 2)]

            def out_proj(nn):
                for w in out_proj_thunks(nn):
                    w()

            prevpair = None
            carry = []
            last_op = []
            pending_st = []
            for pos in range(NMC):
                n = order[pos]
                for u in range(2):
                    work = []
                    if prevpair is not None:
                        ppos, pn, pu, pA, pB = prevpair
                        work = normalize_pair(pn, pu, pA, pB)
                    work += carry
                    carry = []
                    while fill and fill[0][0] <= pos:
                        fill.popleft()[1]()
                    psoA, psoB, pending_st = s_loop_pair(n, u, work, fill,
                                                         pending_st)
                    if prevpair is not None and pu == 1:
                        if ppos == NMC - 2:
                            last_op = out_proj_thunks(pn)
                        else:
                            carry = out_proj_thunks(pn)
                    prevpair = (pos, n, u, psoA, psoB)
            # tail: the last pair's deferred stiles complete its pso first
            for idx in range(len(pending_st)):
                pending_st[idx]()
                if carry:
                    carry.pop(0)()
            for w in carry:
                w()
            ppos, pn, pu, pA, pB = prevpair
            wAB = normalize_pair(pn, pu, pA, pB)
            lo = last_op if NMC >= 2 else []
            for idx in range(max(len(wAB), len(lo))):
                if idx < len(wAB):
                    wAB[idx]()
                if idx < len(lo):
                    lo[idx]()
            out_proj(pn)

    nc.compile()
    return nc


def _get_program(T_, cls):
    key = (T_, tuple(map(tuple, cls.tolist())))
    if key not in _prog_cache:
        _prog_cache[key] = _build(T_, key[1])
    return _prog_cache[key]


def _numpy_ref(query, attn_mask, key_padding_mask, Wq, bq, Wk, bk, Wv, bv,
               Wo, bo):
    """Exact-semantics fallback (mirrors reference.py in numpy)."""
    q = (query @ Wq.T + bq) * SCALE
    k = query @ Wk.T + bk
    v = query @ Wv.T + bv

    def shp(x):
        return x.reshape(T, B * H, HD).transpose(1, 0, 2)

    q, k, v = shp(q), shp(k), shp(v)
    w = np.einsum('bth,bsh->bts', q, k).reshape(B, H, T, T) + attn_mask
    w = np.where(key_padding_mask[:, None, None, :], -np.inf, w)
    w = w - w.max(axis=-1, keepdims=True)
    ew = np.exp(w)
    p = (ew / ew.sum(axis=-1, keepdims=True)).reshape(B * H, T, T)
    o = np.einsum('bts,bsh->bth', p, v.reshape(B * H, T, HD))
    o = o.transpose(1, 0, 2).reshape(T, B, E)
    return (o @ Wo.T + bo).astype(np.float32)


def _prep_inputs(query, attn_mask, Wq, bq, Wk, Wv, Wo, cls):
    """Build the 8 per-core input maps."""
    bf = np.float16
    f8np = ml_dtypes.float8_e4m3
    add_blocks = [(mb, sb) for mb in range(T // 128) for sb in range(T // 128)
                  if cls[mb, sb] == ADD]
    n_add = len(add_blocks)
    if n_add:
        mskp = np.empty((128, n_add * 128), np.float32)
        for i, (mb, sb) in enumerate(add_blocks):
            blk = attn_mask[mb * 128:(mb + 1) * 128, sb * 128:(sb + 1) * 128]
            mskp[:, i * 128:(i + 1) * 128] = np.ascontiguousarray(blk.T)
    else:
        mskp = np.zeros((128, 128), np.float32)
    bin_blocks = [(mb, sb) for mb in range(T // 128) for sb in range(T // 128)
                  if cls[mb, sb] == ADDBIN]
    if bin_blocks:
        trip = np.empty((128, len(bin_blocks) * 128), bf)
        for i, (mb, sb) in enumerate(bin_blocks):
            blk = attn_mask[mb * 128:(mb + 1) * 128, sb * 128:(sb + 1) * 128]
            trip[:, i * 128:(i + 1) * 128] = (blk.T == 0.0).astype(bf)
    else:
        trip = np.zeros((128, 128), bf)

    in_maps = []
    for core in range(NCORES):
        b = core // (NCORES // B)
        jsl = slice((core % (NCORES // B)) * J, (core % (NCORES // B)) * J + J)
        EC_, J_ = E // 128, J

        x = np.ascontiguousarray(query[:, b, :].T)  # [E, T] f32
        xTb_c = np.ascontiguousarray(
            x.reshape(EC_, 128, NB, 128).transpose(1, 2, 0, 3)
            .reshape(128, NB * EC_ * 128)).astype(bf)
        xT8_c = np.ascontiguousarray(
            x.reshape(EC2, 2, 128, T).transpose(2, 0, 1, 3)
            .reshape(128, EC2 * 2 * T)).astype(f8np)

        def w8pack(W):
            wt = W[jsl, :].T * np.float32(8.0)  # [E, J], x8 prescale
            return np.ascontiguousarray(
                wt.reshape(EC2, 2, 128, 2, 128).transpose(2, 0, 3, 1, 4)
                .reshape(128, EC2 * 2 * 2 * 128)).astype(f8np)

        def sb_layout(wT):  # [E, J] -> SBUF [128, EC*J]
            return np.ascontiguousarray(
                wT.reshape(EC_, 128, J_).transpose(1, 0, 2)
                .reshape(128, EC_ * J_))

        wq_l = sb_layout((Wq[jsl, :] * np.float32(SCALE)).T)
        wk_l = sb_layout(Wk[jsl, :].T)
        wv_l = sb_layout(Wv[jsl, :].T)

        def usplit(wl, u):  # [128, EC*J] -> u-half [128, EC*128]
            return np.ascontiguousarray(
                wl.reshape(128, EC_, 2, 128)[:, :, u, :]
                .reshape(128, EC_ * 128)).astype(bf)

        woT = Wo[:, jsl].T  # [J, E]
        wopack = np.ascontiguousarray(
            woT.reshape(J_ // 128, 128, E).transpose(1, 0, 2)
            .reshape(128, (J_ // 128) * E)).astype(bf)
        bq_c = np.ascontiguousarray(
            (bq[jsl] * np.float32(SCALE)).reshape(2, 128).T)
        in_maps.append({
            "xTb": xTb_c, "xT8": xT8_c,
            "w8q": w8pack(Wq), "w8k": w8pack(Wk),
            "wqpack0": usplit(wq_l, 0), "wqpack1": usplit(wq_l, 1),
            "wkpack0": usplit(wk_l, 0), "wkpack1": usplit(wk_l, 1),
            "wvpack": np.ascontiguousarray(wv_l).astype(bf),
            "wopack": wopack, "bqp": bq_c, "msk": mskp, "tri": trip,
        })
    return in_maps


def _kernel_impl(inputs, trace=False, **run_kwargs):
    query = np.asarray(inputs["query"], np.float32)
    attn_mask = np.asarray(inputs["attn_mask"], np.float32)
    kpm = np.asarray(inputs["key_padding_mask"])
    Wq = np.asarray(inputs["Wq"], np.float32)
    bq = np.asarray(inputs["bq"], np.float32)
    Wk = np.asarray(inputs["Wk"], np.float32)
    bk = np.asarray(inputs["bk"], np.float32)
    Wv = np.asarray(inputs["Wv"], np.float32)
    bv = np.asarray(inputs["bv"], np.float32)
    Wo = np.asarray(inputs["Wo"], np.float32)
    bo = np.asarray(inputs["bo"], np.float32)

    cls = _classify_mask(attn_mask)
    fallback = (
        kpm.any()
        or (attn_mask.max(axis=1) <= NEG_THRESH).any()
        or (cls == ADD).sum() > 24 or (cls == ADDBIN).sum() > 24
        or np.isnan(attn_mask).any()
    )
    if fallback:
        return _numpy_ref(query, attn_mask, kpm, Wq, bq, Wk, bk, Wv, bv,
                          Wo, bo), None

    nc = _get_program(T, cls)
    in_maps = _prep_inputs(query, attn_mask, Wq, bq, Wk, Wv, Wo, cls)
    for attempt in range(3):
        res = run_bass_kernel_spmd(nc, in_maps, core_ids=list(range(NCORES)),
                                   trace=trace, **run_kwargs)
        if all(np.isfinite(r["out"]).all() for r in res.results):
            break
    else:
        return _numpy_ref(query, attn_mask, kpm, Wq, bq, Wk, bk, Wv, bv,
                          Wo, bo), None

    # unshard: sum the 4 row-split partials per batch element (the Wo
    # all-reduce), then add bo and the bv contribution (sum_s p = 1).
    bo_total = bo + Wo @ bv
    out = np.empty((T, B, E), np.float32)
    gsz = NCORES // B
    for b in range(B):
        acc = res.results[b * gsz]["out"].astype(np.float32)
        for c in range(b * gsz + 1, (b + 1) * gsz):
            acc = acc + res.results[c]["out"].astype(np.float32)
        out[:, b, :] = acc + bo_total[None, :]
    return out, res


def kernel(**inputs):
    out, _ = _kernel_impl(inputs, trace=False)
    return out
